# revision 6
# baseline (speedup 1.0000x reference)
"""Trainium2 Bass kernel for CausalWaveletFieldAttention (v2, fp16 datapath).

Shapes (hardcoded): x [B=4, N=4096, D=1024], H=16 heads, HD=64.
Sharding over 8 cores: core c handles (batch b = c//2, half = c%2), i.e.
2048 contiguous sequence rows of one batch.

Per-core pipeline, all in transposed [channel, seq] layout, fp16 SBUF
datapath (output tolerance is 2e-2; fp16 keeps us ~1e-3):
  1. k/v projections on PE (fp16 matmuls, k-outer loop so each stationary
     strip is reused across 4 PSUM banks), Act evicts with Square/Identity
  2. k_mag via PE block-ones reduce + Act Sqrt + PE broadcast,
     f0 = v * k_mag fused on DVE (4x fp16 mode)
  3. pairwise fp16 AllGather of f0 halves (causal-conv history)
  4. gate = sigmoid(x @ (Wq@Wgate) + b') on PE, Act Sigmoid evict,
     kept in SBUF (no DRAM round trip)
  5. 24-tap dilated wavelet FIR entirely on DVE as scalar_tensor_tensor
     chains over [128, 3072] fp16 tiles (4x packed mode, ~800ns/tap),
     extended 1024 cols back so the d=512/1024 skip taps apply locally;
     overlaps the gate matmuls on PE
  6. head coupling as dense [1024,1024] fp16 matmul on PE; Act evicts,
     DVE multiplies by the SBUF-resident gate (4x)
  7. out = (field*gate) @ Wout + bout with pg strips as PE stationaries
     straight from SBUF (no transpose round trip)
"""

import numpy as np

import concourse.bass as bass
import concourse.mybir as mybir
import concourse.tile as tile
from concourse import bacc
from concourse.bass_utils import run_bass_kernel_spmd

F32 = mybir.dt.float32
F16 = mybir.dt.float16
AF = mybir.ActivationFunctionType
ALU = mybir.AluOpType

B, N, D, H, HD = 4, 4096, 1024, 16, 64
NCORES = 8
SEQ = N // 2          # 2048 rows per core
KC = D // 128         # 8 contraction chunks
CONVN = SEQ + 1024    # 3072 conv outputs (1024 extra for skip taps)
EXT = SEQ + 4096      # 6144 extended f0 buffer
D4 = [0.4829629131445341, 0.8365163037378079, 0.2241438680420134, -0.1294095225512604]
N_SCALES = 11
SPARSE_DILATIONS = (512, 1024)
SHIFTS = [0, 1, 2, 3, 4, 6, 8, 12, 16, 24, 32, 48, 64, 96, 128, 192, 256,
          384, 512, 768, 1024, 1536, 2048, 3072]
NT = len(SHIFTS)      # 24 taps

_PROGRAM_CACHE = {}


def _build_program():
    if "p" in _PROGRAM_CACHE:
        return _PROGRAM_CACHE["p"]

    nc = bacc.Bacc("TRN2", target_bir_lowering=False, debug=False,
                   num_devices=NCORES)

    # ---- parameters (per-core); weights pre-packed partition-major ----
    xm_in = nc.declare_dram_parameter("xm_in", [128, KC, SEQ], F16, isOutput=False)
    mask = nc.declare_dram_parameter("mask", [128, 1], F32, isOutput=False)
    WkS = nc.declare_dram_parameter("WkS", [128, KC, KC, 128], F16, isOutput=False)
    WvS = nc.declare_dram_parameter("WvS", [128, KC, KC, 128], F16, isOutput=False)
    WgS = nc.declare_dram_parameter("WgS", [128, KC, KC, 128], F16, isOutput=False)
    McS = nc.declare_dram_parameter("McS", [128, KC, KC, 128], F16, isOutput=False)
    WoT = nc.declare_dram_parameter("WoT", [128, KC, D], F16, isOutput=False)
    bkT = nc.declare_dram_parameter("bkT", [128, KC], F32, isOutput=False)
    bvT = nc.declare_dram_parameter("bvT", [128, KC], F32, isOutput=False)
    bgT = nc.declare_dram_parameter("bgT", [128, KC], F32, isOutput=False)
    boutB = nc.declare_dram_parameter("boutB", [128, D], F32, isOutput=False)
    wchan = nc.declare_dram_parameter("wchan", [128, KC, NT], F32, isOutput=False)
    swt = nc.declare_dram_parameter("swt", [128, 2], F32, isOutput=False)
    bo_in = nc.declare_dram_parameter("bo_in", [128, 2], F16, isOutput=False)
    on_in = nc.declare_dram_parameter("on_in", [2, 128], F16, isOutput=False)
    out = nc.declare_dram_parameter("out", [SEQ, D], F32, isOutput=True)

    # ---- internal DRAM (collective staging) ----
    f0_dram = [nc.dram_tensor(f"f0_dram{c}", [128, SEQ], F16)
               for c in range(KC)]
    f0_gath = [nc.dram_tensor(f"f0_gath{c}", [2, 128, SEQ], F16)
               for c in range(KC)]

    with tile.TileContext(nc) as tc:
        with (
            tc.tile_pool(name="psum", bufs=1, space="PSUM") as psp,
            tc.tile_pool(name="const", bufs=1) as constp,
            tc.tile_pool(name="big", bufs=1) as bigp,
        ):
            # ---- constants ----
            bo_t = constp.tile([128, 2], F16)
            nc.sync.dma_start(bo_t[:], bo_in[:])
            on_t = constp.tile([2, 128], F16)
            nc.sync.dma_start(on_t[:], on_in[:])
            wchan_t = constp.tile([128, KC, NT], F32)
            nc.sync.dma_start(wchan_t[:], wchan[:])
            swt_t = constp.tile([128, 2], F32)
            nc.sync.dma_start(swt_t[:], swt[:])
            bk_t = constp.tile([128, KC], F32)
            nc.sync.dma_start(bk_t[:], bkT[:])
            bv_t = constp.tile([128, KC], F32)
            nc.sync.dma_start(bv_t[:], bvT[:])
            bg_t = constp.tile([128, KC], F32)
            nc.sync.dma_start(bg_t[:], bgT[:])
            mask_t = constp.tile([128, 1], F32)
            nc.sync.dma_start(mask_t[:], mask[:])

            gate_t = bigp.tile([128, KC, SEQ], F16, tag="gate")
            field = bigp.tile([128, KC, SEQ], F16, tag="field")

            def psum_tile(tag, bufs):
                return psp.tile([128, 512], F32, tag=tag, bufs=bufs,
                                name=tag)

            # ======== phase B (k/v/f0/gate) + phase C (conv), one scope
            # so conv overlaps the gate matmuls without pool aliasing ====
            with (
                tc.tile_pool(name="p_xm", bufs=1) as p_xm,
                tc.tile_pool(name="p_str", bufs=3) as p_str,
                tc.tile_pool(name="p_ev", bufs=2) as p_ev,
                tc.tile_pool(name="p_cw", bufs=1) as p_cw,
            ):
                xm = p_xm.tile([128, KC, SEQ], F16, tag="xm")
                nc.sync.dma_start(xm[:], xm_in[:])

                def strip_matmuls(sr, evict):
                    """8 stationary chunks x 4 psum banks; evict(rb, psum)."""
                    pss = [psum_tile(f"ps{rb}", 2 if rb < 2 else 1)
                           for rb in range(4)]
                    for k in range(KC):
                        for rb in range(4):
                            nc.tensor.matmul(
                                pss[rb][:], sr[:, k, :],
                                xm[:, k, rb * 512:(rb + 1) * 512],
                                start=(k == 0), stop=(k == KC - 1))
                    for rb in range(4):
                        evict(rb, pss[rb])

                for c in range(KC):
                    ks = p_str.tile([128, KC, 128], F16, tag="strip")
                    nc.sync.dma_start(ks[:], WkS[:, c, :, :])
                    k2b = p_ev.tile([128, SEQ], F16, tag="k2b")
                    strip_matmuls(
                        ks, lambda rb, ps: nc.scalar.activation(
                            k2b[:, rb * 512:(rb + 1) * 512], ps[:],
                            AF.Square, bias=bk_t[:, c:c + 1]))
                    vs = p_str.tile([128, KC, 128], F16, tag="strip")
                    nc.sync.dma_start(vs[:], WvS[:, c, :, :])
                    vTb = p_ev.tile([128, SEQ], F16, tag="vTb")
                    strip_matmuls(
                        vs, lambda rb, ps: nc.scalar.activation(
                            vTb[:, rb * 512:(rb + 1) * 512], ps[:],
                            AF.Identity, bias=bv_t[:, c:c + 1]))
                    km = p_ev.tile([2, SEQ], F16, tag="km")
                    for sb in range(4):
                        pss = psp.tile([2, 512], F32, tag="km2", bufs=1,
                                       name="km2")
                        nc.tensor.matmul(pss[:], bo_t[:],
                                         k2b[:, sb * 512:(sb + 1) * 512],
                                         start=True, stop=True)
                        nc.scalar.activation(km[:, sb * 512:(sb + 1) * 512],
                                             pss[:], AF.Sqrt)
                    kmagb = p_ev.tile([128, SEQ], F16, tag="kmagb")
                    for sb in range(4):
                        pse = psum_tile("psb", 1)
                        nc.tensor.matmul(pse[:], on_t[:],
                                         km[:, sb * 512:(sb + 1) * 512],
                                         start=True, stop=True)
                        nc.scalar.activation(
                            kmagb[:, sb * 512:(sb + 1) * 512], pse[:],
                            AF.Identity)
                    # f0 = v * k_mag on DVE (all fp16 SBUF -> 4x mode)
                    f0b = p_ev.tile([128, SEQ], F16, tag="f0b")
                    nc.vector.scalar_tensor_tensor(
                        f0b[:], vTb[:], 1.0, kmagb[:],
                        op0=ALU.mult, op1=ALU.mult)
                    nc.sync.dma_start(f0_dram[c][:], f0b[:])
                    nc.gpsimd.collective_compute(
                        "AllGather", ALU.bypass,
                        replica_groups=[[0, 1], [2, 3], [4, 5], [6, 7]],
                        ins=[f0_dram[c][:]], outs=[f0_gath[c][:]])

                # gate = sigmoid(x @ (Wq@Wgate) + b'), stays in SBUF
                for gc in range(KC):
                    gs = p_str.tile([128, KC, 128], F16, tag="strip")
                    nc.sync.dma_start(gs[:], WgS[:, gc, :, :])
                    strip_matmuls(
                        gs, lambda rb, ps: nc.scalar.activation(
                            gate_t[:, gc, rb * 512:(rb + 1) * 512], ps[:],
                            AF.Sigmoid, bias=bg_t[:, gc:gc + 1]))

                # ---- phase C: wavelet FIR + skips (DVE, overlaps gate) ----
                exts = []
                for i in range(2):
                    e = p_cw.tile([128, EXT], F16, tag=f"ext{i}")
                    nc.vector.memset(e[:, 0:2048], 0.0)
                    exts.append(e)
                acc = [p_cw.tile([128, CONVN], F16, tag=f"acc{j}", name=f"acc{j}")
                       for j in range(2)]
                tmp = p_cw.tile([128, SEQ], F16, tag="tmp")
                for c in range(KC):
                    ext = exts[c % 2]
                    halo = p_cw.tile([128, SEQ], F16, tag="halo", bufs=2)
                    nc.sync.dma_start(halo[:], f0_gath[c][0, :, :])
                    nc.vector.tensor_scalar_mul(ext[:, 2048:4096], halo[:],
                                                mask_t[:, 0:1])
                    nc.sync.dma_start(ext[:, 4096:EXT], f0_dram[c][:])
                    cur = None
                    for si, s in enumerate(SHIFTS):
                        src = ext[:, 3072 - s:3072 - s + CONVN]
                        w = wchan_t[:, c, si:si + 1]
                        if cur is None:
                            cur = acc[0]
                            nc.vector.tensor_scalar_mul(cur[:], src, w)
                        else:
                            nxt = acc[si % 2]
                            nc.vector.scalar_tensor_tensor(
                                nxt[:], src, w, cur[:],
                                op0=ALU.mult, op1=ALU.add)
                            cur = nxt
                    # skips: field[n] = conv[n] + sw0*conv[n-512] + sw1*conv[n-1024]
                    nc.vector.scalar_tensor_tensor(
                        tmp[:], cur[:, 512:512 + SEQ], swt_t[:, 0:1],
                        cur[:, 1024:1024 + SEQ],
                        op0=ALU.mult, op1=ALU.add)
                    nc.vector.scalar_tensor_tensor(
                        field[:, c, :], cur[:, 0:SEQ], swt_t[:, 1:2],
                        tmp[:],
                        op0=ALU.mult, op1=ALU.add)

            # ================= phase E: coupling + gate =================
            with (
                tc.tile_pool(name="p_mc", bufs=1) as p_mc,
                tc.tile_pool(name="p_pg", bufs=1) as p_pg,
                tc.tile_pool(name="p_ev2", bufs=3) as p_ev2,
                tc.tile_pool(name="p_wo", bufs=1) as p_wo,
                tc.tile_pool(name="p_fw", bufs=3) as p_fw,
            ):
                mc_all = p_mc.tile([128, KC, KC, 128], F16, tag="mc")
                nc.sync.dma_start(mc_all[:], McS[:])
                wo_all = p_wo.tile([128, KC, D], F16, tag="wo")
                nc.sync.dma_start(wo_all[:], WoT[:])
                bout_t = p_wo.tile([128, D], F32, tag="bout")
                nc.sync.dma_start(bout_t[:], boutB[:])
                pg = p_pg.tile([128, KC, SEQ], F16, tag="pg")
                for co in range(KC):
                    pss = [psum_tile(f"ps{sb}", 2 if sb < 2 else 1)
                           for sb in range(4)]
                    for ci in range(KC):
                        for sb in range(4):
                            nc.tensor.matmul(
                                pss[sb][:], mc_all[:, co, ci, :],
                                field[:, ci, sb * 512:(sb + 1) * 512],
                                start=(ci == 0), stop=(ci == KC - 1))
                    for sb in range(4):
                        cpl = p_ev2.tile([128, 512], F16, tag="cpl")
                        nc.scalar.activation(cpl[:], pss[sb][:], AF.Identity)
                        nc.vector.scalar_tensor_tensor(
                            pg[:, co, sb * 512:(sb + 1) * 512],
                            gate_t[:, co, sb * 512:(sb + 1) * 512], 1.0,
                            cpl[:], op0=ALU.mult, op1=ALU.mult)

                # ---- phase F: out = pg @ Wout + bout ----
                for st in range(SEQ // 128):
                    pso = [psum_tile(f"ps{cb}", 2) for cb in range(2)]
                    for k in range(KC):
                        for cb in range(2):
                            nc.tensor.matmul(
                                pso[cb][:],
                                pg[:, k, st * 128:(st + 1) * 128],
                                wo_all[:, k, cb * 512:(cb + 1) * 512],
                                start=(k == 0), stop=(k == KC - 1))
                    outb = p_fw.tile([128, D], F32, tag="outb")
                    for cb in range(2):
                        nc.vector.tensor_add(
                            outb[:, cb * 512:(cb + 1) * 512], pso[cb][:],
                            bout_t[:, cb * 512:(cb + 1) * 512])
                    nc.sync.dma_start(out[st * 128:(st + 1) * 128, :],
                                      outb[:])

    nc.compile()
    _PROGRAM_CACHE["p"] = nc
    return nc


def _softmax(a, axis):
    a = a - a.max(axis=axis, keepdims=True)
    e = np.exp(a)
    return e / e.sum(axis=axis, keepdims=True)


def _host_prep(inputs):
    """Build per-core and replicated input tensors from full inputs."""
    x = np.asarray(inputs["x"], np.float32)
    Wqkv = np.asarray(inputs["Wqkv"], np.float32)
    bqkv = np.asarray(inputs["bqkv"], np.float32)
    Wout = np.asarray(inputs["Wout"], np.float32)
    bout = np.asarray(inputs["bout"], np.float32)
    Wgate = np.asarray(inputs["Wgate"], np.float32)
    bgate = np.asarray(inputs["bgate"], np.float32)
    scale_gain = np.asarray(inputs["scale_gain"], np.float64)
    skip_w = np.asarray(inputs["skip_w"], np.float64)
    coupling = np.asarray(inputs["coupling"], np.float64)

    gains = _softmax(scale_gain, axis=0)              # [11, H]
    sw = 1.0 / (1.0 + np.exp(-skip_w))                # [2]
    coup = _softmax(coupling, axis=-1)                # [H, H]

    sidx = {s: i for i, s in enumerate(SHIFTS)}
    wtab = np.zeros((NT, H), np.float64)
    for j in range(N_SCALES):
        d = 1 << j
        for t in range(4):
            wtab[sidx[(3 - t) * d]] += D4[t] * gains[j]
    ch = np.arange(D)
    wchan = np.zeros((128, KC, NT), np.float32)
    for c in range(KC):
        heads = (ch[c * 128:(c + 1) * 128] // HD)
        wchan[:, c, :] = wtab[:, heads].T.astype(np.float32)

    Mc = np.zeros((D, D), np.float32)
    idx = np.arange(HD)
    for i in range(H):
        for j in range(H):
            Mc[j * HD + idx, i * HD + idx] = coup[i, j]

    # fold the q projection into the gate: gate = sigmoid(x @ (Wq@Wgate) + b')
    Wq = Wqkv[:, :D].astype(np.float64)
    Wqg = (Wq @ Wgate.astype(np.float64)).astype(np.float32)
    bg_f = (bqkv[:D].astype(np.float64) @ Wgate.astype(np.float64)
            + bgate.astype(np.float64)).astype(np.float32)

    def strips(W):
        """[D, D] weight -> [128, KC(strip), KC(contract), 128] fp16."""
        # W[kc*128+p, s*128+j] -> out[p, s, kc, j]
        t = W.reshape(KC, 128, KC, 128)               # [kc, p, s, j]
        return np.ascontiguousarray(
            t.transpose(1, 2, 0, 3).astype(np.float16))

    WkS = strips(Wqkv[:, D:2 * D])
    WvS = strips(Wqkv[:, 2 * D:3 * D])
    WgS = strips(Wqg)
    McS = strips(Mc)
    # Wout moving layout: [p, k, m] = Wout[k*128+p, m]
    WoT = np.ascontiguousarray(
        Wout.reshape(KC, 128, D).transpose(1, 0, 2).astype(np.float16))

    bkT = bqkv[D:2 * D].reshape(KC, 128).T.copy()     # [128, KC]
    bvT = bqkv[2 * D:3 * D].reshape(KC, 128).T.copy()
    bgT = bg_f.reshape(KC, 128).T.copy()
    boutB = np.broadcast_to(bout, (128, D)).copy()
    swt = np.broadcast_to(sw.astype(np.float32), (128, 2)).copy()
    bo = np.zeros((128, 2), np.float16)
    bo[0:64, 0] = 1.0
    bo[64:128, 1] = 1.0
    on = np.zeros((2, 128), np.float16)
    on[0, 0:64] = 1.0
    on[1, 64:128] = 1.0

    shared = dict(WkS=WkS, WvS=WvS, WgS=WgS, McS=McS, WoT=WoT,
                  bkT=bkT, bvT=bvT, bgT=bgT, boutB=boutB, wchan=wchan,
                  swt=swt, bo_in=bo, on_in=on)
    in_maps = []
    for c in range(NCORES):
        b, half = c // 2, c % 2
        g0 = half * SEQ
        # xm[p, kc, n] = x[b, g0+n, kc*128+p]
        xc = x[b, g0:g0 + SEQ, :].reshape(SEQ, KC, 128)
        xm = np.ascontiguousarray(
            xc.transpose(2, 1, 0).astype(np.float16))
        m = np.full((128, 1), float(half), np.float32)
        in_maps.append(dict(xm_in=xm, mask=m, **shared))
    return in_maps


def run_cores(inputs, debug_outputs=False, trace=False):
    nc = _build_program()
    in_maps = _host_prep(inputs)
    res = run_bass_kernel_spmd(nc, in_maps, list(range(NCORES)), trace=trace)
    return res


def kernel(**inputs) -> np.ndarray:
    res = run_cores(inputs)
    out = np.empty((B, N, D), np.float32)
    for c in range(NCORES):
        b, half = c // 2, c % 2
        out[b, half * SEQ:(half + 1) * SEQ, :] = res.results[c]["out"]
    return out


# revision 7
# speedup vs baseline: 1.0004x; 1.0004x over previous
"""Trainium2 Bass kernel for CausalWaveletFieldAttention (v2, fp16 datapath).

Shapes (hardcoded): x [B=4, N=4096, D=1024], H=16 heads, HD=64.
Sharding over 8 cores: core c handles (batch b = c//2, half = c%2), i.e.
2048 contiguous sequence rows of one batch.

Per-core pipeline, all in transposed [channel, seq] layout, fp16 SBUF
datapath (output tolerance is 2e-2; fp16 keeps us ~1e-3):
  1. k/v projections on PE (fp16 matmuls, k-outer loop so each stationary
     strip is reused across 4 PSUM banks), Act evicts with Square/Identity
  2. k_mag via PE block-ones reduce + Act Sqrt + PE broadcast,
     f0 = v * k_mag fused on DVE (4x fp16 mode)
  3. pairwise fp16 AllGather of f0 halves (causal-conv history)
  4. gate = sigmoid(x @ (Wq@Wgate) + b') on PE, Act Sigmoid evict,
     kept in SBUF (no DRAM round trip)
  5. 24-tap dilated wavelet FIR entirely on DVE as scalar_tensor_tensor
     chains over [128, 3072] fp16 tiles (4x packed mode, ~800ns/tap),
     extended 1024 cols back so the d=512/1024 skip taps apply locally;
     overlaps the gate matmuls on PE
  6. head coupling as dense [1024,1024] fp16 matmul on PE; Act evicts,
     DVE multiplies by the SBUF-resident gate (4x)
  7. out = (field*gate) @ Wout + bout with pg strips as PE stationaries
     straight from SBUF (no transpose round trip)
"""

import numpy as np
import ml_dtypes

import concourse.bass as bass
import concourse.mybir as mybir
import concourse.tile as tile
from concourse import bacc
from concourse.bass_utils import run_bass_kernel_spmd

F32 = mybir.dt.float32
F16 = mybir.dt.float16
BF16 = mybir.dt.bfloat16
AF = mybir.ActivationFunctionType
ALU = mybir.AluOpType

B, N, D, H, HD = 4, 4096, 1024, 16, 64
NCORES = 8
SEQ = N // 2          # 2048 rows per core
KC = D // 128         # 8 contraction chunks
CONVN = SEQ + 1024    # 3072 conv outputs (1024 extra for skip taps)
EXT = SEQ + 4096      # 6144 extended f0 buffer
D4 = [0.4829629131445341, 0.8365163037378079, 0.2241438680420134, -0.1294095225512604]
N_SCALES = 11
SPARSE_DILATIONS = (512, 1024)
SHIFTS = [0, 1, 2, 3, 4, 6, 8, 12, 16, 24, 32, 48, 64, 96, 128, 192, 256,
          384, 512, 768, 1024, 1536, 2048, 3072]
NT = len(SHIFTS)      # 24 taps

_PROGRAM_CACHE = {}


def _build_program():
    if "p" in _PROGRAM_CACHE:
        return _PROGRAM_CACHE["p"]

    nc = bacc.Bacc("TRN2", target_bir_lowering=False, debug=False,
                   num_devices=NCORES)

    # ---- parameters (per-core); weights pre-packed partition-major ----
    xm_in = nc.declare_dram_parameter("xm_in", [128, KC, SEQ], F16, isOutput=False)
    mask = nc.declare_dram_parameter("mask", [128, 1], F32, isOutput=False)
    WkS = nc.declare_dram_parameter("WkS", [128, KC, KC, 128], F16, isOutput=False)
    WvS = nc.declare_dram_parameter("WvS", [128, KC, KC, 128], F16, isOutput=False)
    WgS = nc.declare_dram_parameter("WgS", [128, KC, KC, 128], F16, isOutput=False)
    McS = nc.declare_dram_parameter("McS", [128, KC, KC, 128], BF16, isOutput=False)
    WoT = nc.declare_dram_parameter("WoT", [128, KC, D], F16, isOutput=False)
    bkT = nc.declare_dram_parameter("bkT", [128, KC], F32, isOutput=False)
    bvT = nc.declare_dram_parameter("bvT", [128, KC], F32, isOutput=False)
    bgT = nc.declare_dram_parameter("bgT", [128, KC], F32, isOutput=False)
    boutB = nc.declare_dram_parameter("boutB", [128, D], F32, isOutput=False)
    wchan = nc.declare_dram_parameter("wchan", [128, KC, NT], F32, isOutput=False)
    swt = nc.declare_dram_parameter("swt", [128, 2], F32, isOutput=False)
    bo_in = nc.declare_dram_parameter("bo_in", [128, 2], F16, isOutput=False)
    on_in = nc.declare_dram_parameter("on_in", [2, 128], F16, isOutput=False)
    out = nc.declare_dram_parameter("out", [SEQ, D], F32, isOutput=True)

    # ---- internal DRAM (collective staging) ----
    f0_dram = [nc.dram_tensor(f"f0_dram{c}", [128, SEQ], BF16)
               for c in range(KC)]
    f0_gath = [nc.dram_tensor(f"f0_gath{c}", [2, 128, SEQ], BF16)
               for c in range(KC)]

    with tile.TileContext(nc) as tc:
        with (
            tc.tile_pool(name="psum", bufs=1, space="PSUM") as psp,
            tc.tile_pool(name="const", bufs=1) as constp,
            tc.tile_pool(name="big", bufs=1) as bigp,
        ):
            # ---- constants ----
            bo_t = constp.tile([128, 2], F16)
            nc.sync.dma_start(bo_t[:], bo_in[:])
            on_t = constp.tile([2, 128], F16)
            nc.sync.dma_start(on_t[:], on_in[:])
            wchan_t = constp.tile([128, KC, NT], F32)
            nc.sync.dma_start(wchan_t[:], wchan[:])
            swt_t = constp.tile([128, 2], F32)
            nc.sync.dma_start(swt_t[:], swt[:])
            bk_t = constp.tile([128, KC], F32)
            nc.sync.dma_start(bk_t[:], bkT[:])
            bv_t = constp.tile([128, KC], F32)
            nc.sync.dma_start(bv_t[:], bvT[:])
            bg_t = constp.tile([128, KC], F32)
            nc.sync.dma_start(bg_t[:], bgT[:])
            mask_t = constp.tile([128, 1], F32)
            nc.sync.dma_start(mask_t[:], mask[:])

            gate_t = bigp.tile([128, KC, SEQ], F16, tag="gate")
            field = bigp.tile([128, KC, SEQ], BF16, tag="field")

            def psum_tile(tag, bufs):
                return psp.tile([128, 512], F32, tag=tag, bufs=bufs,
                                name=tag)

            # ======== phase B (k/v/f0/gate) + phase C (conv), one scope
            # so conv overlaps the gate matmuls without pool aliasing ====
            with (
                tc.tile_pool(name="p_xm", bufs=1) as p_xm,
                tc.tile_pool(name="p_str", bufs=3) as p_str,
                tc.tile_pool(name="p_ev", bufs=2) as p_ev,
                tc.tile_pool(name="p_cw", bufs=1) as p_cw,
            ):
                xm = p_xm.tile([128, KC, SEQ], F16, tag="xm")
                nc.sync.dma_start(xm[:], xm_in[:])

                def strip_matmuls(sr, evict):
                    """8 stationary chunks x 4 psum banks; evict(rb, psum)."""
                    pss = [psum_tile(f"ps{rb}", 2 if rb < 2 else 1)
                           for rb in range(4)]
                    for k in range(KC):
                        for rb in range(4):
                            nc.tensor.matmul(
                                pss[rb][:], sr[:, k, :],
                                xm[:, k, rb * 512:(rb + 1) * 512],
                                start=(k == 0), stop=(k == KC - 1))
                    for rb in range(4):
                        evict(rb, pss[rb])

                for c in range(KC):
                    ks = p_str.tile([128, KC, 128], F16, tag="strip")
                    nc.sync.dma_start(ks[:], WkS[:, c, :, :])
                    k2b = p_ev.tile([128, SEQ], F16, tag="k2b")
                    strip_matmuls(
                        ks, lambda rb, ps: nc.scalar.activation(
                            k2b[:, rb * 512:(rb + 1) * 512], ps[:],
                            AF.Square, bias=bk_t[:, c:c + 1]))
                    vs = p_str.tile([128, KC, 128], F16, tag="strip")
                    nc.sync.dma_start(vs[:], WvS[:, c, :, :])
                    vTb = p_ev.tile([128, SEQ], F16, tag="vTb")
                    strip_matmuls(
                        vs, lambda rb, ps: nc.scalar.activation(
                            vTb[:, rb * 512:(rb + 1) * 512], ps[:],
                            AF.Identity, bias=bv_t[:, c:c + 1]))
                    km = p_ev.tile([2, SEQ], F16, tag="km")
                    for sb in range(4):
                        pss = psp.tile([2, 512], F32, tag="km2", bufs=1,
                                       name="km2")
                        nc.tensor.matmul(pss[:], bo_t[:],
                                         k2b[:, sb * 512:(sb + 1) * 512],
                                         start=True, stop=True)
                        nc.scalar.activation(km[:, sb * 512:(sb + 1) * 512],
                                             pss[:], AF.Sqrt)
                    kmagb = p_ev.tile([128, SEQ], F16, tag="kmagb")
                    for sb in range(4):
                        pse = psum_tile("psb", 1)
                        nc.tensor.matmul(pse[:], on_t[:],
                                         km[:, sb * 512:(sb + 1) * 512],
                                         start=True, stop=True)
                        nc.scalar.activation(
                            kmagb[:, sb * 512:(sb + 1) * 512], pse[:],
                            AF.Identity)
                    # f0 = v * k_mag on DVE (all fp16 SBUF -> 4x mode)
                    f0b = p_ev.tile([128, SEQ], BF16, tag="f0b")
                    nc.vector.scalar_tensor_tensor(
                        f0b[:], vTb[:], 1.0, kmagb[:],
                        op0=ALU.mult, op1=ALU.mult)
                    nc.sync.dma_start(f0_dram[c][:], f0b[:])
                    nc.gpsimd.collective_compute(
                        "AllGather", ALU.bypass,
                        replica_groups=[[0, 1], [2, 3], [4, 5], [6, 7]],
                        ins=[f0_dram[c][:]], outs=[f0_gath[c][:]])

                # gate = sigmoid(x @ (Wq@Wgate) + b'), stays in SBUF
                for gc in range(KC):
                    gs = p_str.tile([128, KC, 128], F16, tag="strip")
                    nc.sync.dma_start(gs[:], WgS[:, gc, :, :])
                    strip_matmuls(
                        gs, lambda rb, ps: nc.scalar.activation(
                            gate_t[:, gc, rb * 512:(rb + 1) * 512], ps[:],
                            AF.Sigmoid, bias=bg_t[:, gc:gc + 1]))

                # ---- phase C: wavelet FIR + skips (DVE, overlaps gate) ----
                exts = []
                for i in range(2):
                    e = p_cw.tile([128, EXT], BF16, tag=f"ext{i}")
                    nc.vector.memset(e[:, 0:2048], 0.0)
                    exts.append(e)
                acc = [p_cw.tile([128, CONVN], BF16, tag=f"acc{j}", name=f"acc{j}")
                       for j in range(2)]
                tmp = p_cw.tile([128, SEQ], BF16, tag="tmp")
                for c in range(KC):
                    ext = exts[c % 2]
                    halo = p_cw.tile([128, SEQ], BF16, tag="halo", bufs=2)
                    nc.sync.dma_start(halo[:], f0_gath[c][0, :, :])
                    nc.vector.tensor_scalar_mul(ext[:, 2048:4096], halo[:],
                                                mask_t[:, 0:1])
                    nc.sync.dma_start(ext[:, 4096:EXT], f0_dram[c][:])
                    cur = None
                    for si, s in enumerate(SHIFTS):
                        src = ext[:, 3072 - s:3072 - s + CONVN]
                        w = wchan_t[:, c, si:si + 1]
                        if cur is None:
                            cur = acc[0]
                            nc.vector.tensor_scalar_mul(cur[:], src, w)
                        else:
                            nxt = acc[si % 2]
                            nc.vector.scalar_tensor_tensor(
                                nxt[:], src, w, cur[:],
                                op0=ALU.mult, op1=ALU.add)
                            cur = nxt
                    # skips: field[n] = conv[n] + sw0*conv[n-512] + sw1*conv[n-1024]
                    nc.vector.scalar_tensor_tensor(
                        tmp[:], cur[:, 512:512 + SEQ], swt_t[:, 0:1],
                        cur[:, 1024:1024 + SEQ],
                        op0=ALU.mult, op1=ALU.add)
                    nc.vector.scalar_tensor_tensor(
                        field[:, c, :], cur[:, 0:SEQ], swt_t[:, 1:2],
                        tmp[:],
                        op0=ALU.mult, op1=ALU.add)

            # ================= phase E: coupling + gate =================
            with (
                tc.tile_pool(name="p_mc", bufs=1) as p_mc,
                tc.tile_pool(name="p_pg", bufs=1) as p_pg,
                tc.tile_pool(name="p_ev2", bufs=3) as p_ev2,
                tc.tile_pool(name="p_wo", bufs=1) as p_wo,
                tc.tile_pool(name="p_fw", bufs=3) as p_fw,
            ):
                mc_all = p_mc.tile([128, KC, KC, 128], BF16, tag="mc")
                nc.sync.dma_start(mc_all[:], McS[:])
                wo_all = p_wo.tile([128, KC, D], F16, tag="wo")
                nc.sync.dma_start(wo_all[:], WoT[:])
                bout_t = p_wo.tile([128, D], F32, tag="bout")
                nc.sync.dma_start(bout_t[:], boutB[:])
                pg = p_pg.tile([128, KC, SEQ], F16, tag="pg")
                for co in range(KC):
                    pss = [psum_tile(f"ps{sb}", 2 if sb < 2 else 1)
                           for sb in range(4)]
                    for ci in range(KC):
                        for sb in range(4):
                            nc.tensor.matmul(
                                pss[sb][:], mc_all[:, co, ci, :],
                                field[:, ci, sb * 512:(sb + 1) * 512],
                                start=(ci == 0), stop=(ci == KC - 1))
                    for sb in range(4):
                        cpl = p_ev2.tile([128, 512], F16, tag="cpl")
                        nc.scalar.activation(cpl[:], pss[sb][:], AF.Identity)
                        nc.vector.scalar_tensor_tensor(
                            pg[:, co, sb * 512:(sb + 1) * 512],
                            gate_t[:, co, sb * 512:(sb + 1) * 512], 1.0,
                            cpl[:], op0=ALU.mult, op1=ALU.mult)

                # ---- phase F: out = pg @ Wout + bout ----
                for st in range(SEQ // 128):
                    pso = [psum_tile(f"ps{cb}", 2) for cb in range(2)]
                    for k in range(KC):
                        for cb in range(2):
                            nc.tensor.matmul(
                                pso[cb][:],
                                pg[:, k, st * 128:(st + 1) * 128],
                                wo_all[:, k, cb * 512:(cb + 1) * 512],
                                start=(k == 0), stop=(k == KC - 1))
                    outb = p_fw.tile([128, D], F32, tag="outb")
                    for cb in range(2):
                        nc.vector.tensor_add(
                            outb[:, cb * 512:(cb + 1) * 512], pso[cb][:],
                            bout_t[:, cb * 512:(cb + 1) * 512])
                    nc.sync.dma_start(out[st * 128:(st + 1) * 128, :],
                                      outb[:])

    nc.compile()
    _PROGRAM_CACHE["p"] = nc
    return nc


def _softmax(a, axis):
    a = a - a.max(axis=axis, keepdims=True)
    e = np.exp(a)
    return e / e.sum(axis=axis, keepdims=True)


def _host_prep(inputs):
    """Build per-core and replicated input tensors from full inputs."""
    x = np.asarray(inputs["x"], np.float32)
    Wqkv = np.asarray(inputs["Wqkv"], np.float32)
    bqkv = np.asarray(inputs["bqkv"], np.float32)
    Wout = np.asarray(inputs["Wout"], np.float32)
    bout = np.asarray(inputs["bout"], np.float32)
    Wgate = np.asarray(inputs["Wgate"], np.float32)
    bgate = np.asarray(inputs["bgate"], np.float32)
    scale_gain = np.asarray(inputs["scale_gain"], np.float64)
    skip_w = np.asarray(inputs["skip_w"], np.float64)
    coupling = np.asarray(inputs["coupling"], np.float64)

    gains = _softmax(scale_gain, axis=0)              # [11, H]
    sw = 1.0 / (1.0 + np.exp(-skip_w))                # [2]
    coup = _softmax(coupling, axis=-1)                # [H, H]

    sidx = {s: i for i, s in enumerate(SHIFTS)}
    wtab = np.zeros((NT, H), np.float64)
    for j in range(N_SCALES):
        d = 1 << j
        for t in range(4):
            wtab[sidx[(3 - t) * d]] += D4[t] * gains[j]
    ch = np.arange(D)
    wchan = np.zeros((128, KC, NT), np.float32)
    for c in range(KC):
        heads = (ch[c * 128:(c + 1) * 128] // HD)
        wchan[:, c, :] = wtab[:, heads].T.astype(np.float32)

    Mc = np.zeros((D, D), np.float32)
    idx = np.arange(HD)
    for i in range(H):
        for j in range(H):
            Mc[j * HD + idx, i * HD + idx] = coup[i, j]

    # fold the q projection into the gate: gate = sigmoid(x @ (Wq@Wgate) + b')
    Wq = Wqkv[:, :D].astype(np.float64)
    Wqg = (Wq @ Wgate.astype(np.float64)).astype(np.float32)
    bg_f = (bqkv[:D].astype(np.float64) @ Wgate.astype(np.float64)
            + bgate.astype(np.float64)).astype(np.float32)

    def strips(W):
        """[D, D] weight -> [128, KC(strip), KC(contract), 128] fp16."""
        # W[kc*128+p, s*128+j] -> out[p, s, kc, j]
        t = W.reshape(KC, 128, KC, 128)               # [kc, p, s, j]
        return np.ascontiguousarray(
            t.transpose(1, 2, 0, 3).astype(np.float16))

    WkS = strips(Wqkv[:, D:2 * D])
    WvS = strips(Wqkv[:, 2 * D:3 * D])
    WgS = strips(Wqg)
    McS = strips(Mc).astype(ml_dtypes.bfloat16)
    # Wout moving layout: [p, k, m] = Wout[k*128+p, m]
    WoT = np.ascontiguousarray(
        Wout.reshape(KC, 128, D).transpose(1, 0, 2).astype(np.float16))

    bkT = bqkv[D:2 * D].reshape(KC, 128).T.copy()     # [128, KC]
    bvT = bqkv[2 * D:3 * D].reshape(KC, 128).T.copy()
    bgT = bg_f.reshape(KC, 128).T.copy()
    boutB = np.broadcast_to(bout, (128, D)).copy()
    swt = np.broadcast_to(sw.astype(np.float32), (128, 2)).copy()
    bo = np.zeros((128, 2), np.float16)
    bo[0:64, 0] = 1.0
    bo[64:128, 1] = 1.0
    on = np.zeros((2, 128), np.float16)
    on[0, 0:64] = 1.0
    on[1, 64:128] = 1.0

    shared = dict(WkS=WkS, WvS=WvS, WgS=WgS, McS=McS, WoT=WoT,
                  bkT=bkT, bvT=bvT, bgT=bgT, boutB=boutB, wchan=wchan,
                  swt=swt, bo_in=bo, on_in=on)
    in_maps = []
    for c in range(NCORES):
        b, half = c // 2, c % 2
        g0 = half * SEQ
        # xm[p, kc, n] = x[b, g0+n, kc*128+p]
        xc = x[b, g0:g0 + SEQ, :].reshape(SEQ, KC, 128)
        xm = np.ascontiguousarray(
            xc.transpose(2, 1, 0).astype(np.float16))
        m = np.full((128, 1), float(half), np.float32)
        in_maps.append(dict(xm_in=xm, mask=m, **shared))
    return in_maps


def run_cores(inputs, debug_outputs=False, trace=False):
    nc = _build_program()
    in_maps = _host_prep(inputs)
    res = run_bass_kernel_spmd(nc, in_maps, list(range(NCORES)), trace=trace)
    return res


def kernel(**inputs) -> np.ndarray:
    res = run_cores(inputs)
    out = np.empty((B, N, D), np.float32)
    for c in range(NCORES):
        b, half = c // 2, c % 2
        out[b, half * SEQ:(half + 1) * SEQ, :] = res.results[c]["out"]
    return out


# revision 8
# speedup vs baseline: 1.4580x; 1.4574x over previous
"""Trainium2 Bass kernel for CausalWaveletFieldAttention (v2, fp16 datapath).

Shapes (hardcoded): x [B=4, N=4096, D=1024], H=16 heads, HD=64.
Sharding over 8 cores: core c handles (batch b = c//2, half = c%2), i.e.
2048 contiguous sequence rows of one batch.

Per-core pipeline, all in transposed [channel, seq] layout, fp16 SBUF
datapath (output tolerance is 2e-2; fp16 keeps us ~1e-3):
  1. k/v projections on PE (fp16 matmuls, k-outer loop so each stationary
     strip is reused across 4 PSUM banks), Act evicts with Square/Identity
  2. k_mag via PE block-ones reduce + Act Sqrt + PE broadcast,
     f0 = v * k_mag fused on DVE (4x fp16 mode)
  3. pairwise fp16 AllGather of f0 halves (causal-conv history)
  4. gate = sigmoid(x @ (Wq@Wgate) + b') on PE, Act Sigmoid evict,
     kept in SBUF (no DRAM round trip)
  5. 24-tap dilated wavelet FIR entirely on DVE as scalar_tensor_tensor
     chains over [128, 3072] fp16 tiles (4x packed mode, ~800ns/tap),
     extended 1024 cols back so the d=512/1024 skip taps apply locally;
     overlaps the gate matmuls on PE
  6. head coupling as dense [1024,1024] fp16 matmul on PE; Act evicts,
     DVE multiplies by the SBUF-resident gate (4x)
  7. out = (field*gate) @ Wout + bout with pg strips as PE stationaries
     straight from SBUF (no transpose round trip)
"""

import numpy as np
import ml_dtypes

import concourse.bass as bass
import concourse.mybir as mybir
import concourse.tile as tile
from concourse import bacc
from concourse.bass_utils import run_bass_kernel_spmd

F32 = mybir.dt.float32
F16 = mybir.dt.float16
BF16 = mybir.dt.bfloat16
AF = mybir.ActivationFunctionType
ALU = mybir.AluOpType

B, N, D, H, HD = 4, 4096, 1024, 16, 64
NCORES = 8
SEQ = N // 2          # 2048 rows per core
KC = D // 128         # 8 contraction chunks
CONVN = SEQ + 1024    # 3072 conv outputs (1024 extra for skip taps)
EXT = SEQ + 4096      # 6144 extended f0 buffer
D4 = [0.4829629131445341, 0.8365163037378079, 0.2241438680420134, -0.1294095225512604]
N_SCALES = 11
SPARSE_DILATIONS = (512, 1024)
SHIFTS = [0, 1, 2, 3, 4, 6, 8, 12, 16, 24, 32, 48, 64, 96, 128, 192, 256,
          384, 512, 768, 1024, 1536, 2048, 3072]
NT = len(SHIFTS)      # 24 taps
DVE_TAPS = [256, 384, 512, 768, 1024, 1536, 2048, 3072]
PE_TAPS = [s for s in SHIFTS if s not in DVE_TAPS]   # 16 taps

_PROGRAM_CACHE = {}


def _build_program():
    if "p" in _PROGRAM_CACHE:
        return _PROGRAM_CACHE["p"]

    nc = bacc.Bacc("TRN2", target_bir_lowering=False, debug=False,
                   num_devices=NCORES)

    # ---- parameters (per-core); weights pre-packed partition-major ----
    xm_in = nc.declare_dram_parameter("xm_in", [128, KC, SEQ], F16, isOutput=False)
    mask = nc.declare_dram_parameter("mask", [128, 1], F32, isOutput=False)
    WkS = nc.declare_dram_parameter("WkS", [128, KC, KC, 128], F16, isOutput=False)
    WvS = nc.declare_dram_parameter("WvS", [128, KC, KC, 128], F16, isOutput=False)
    WgS = nc.declare_dram_parameter("WgS", [128, KC, KC, 128], F16, isOutput=False)
    McS = nc.declare_dram_parameter("McS", [128, KC, KC, 128], F16, isOutput=False)
    WoT = nc.declare_dram_parameter("WoT", [128, KC, D], F16, isOutput=False)
    wdiagS = nc.declare_dram_parameter("wdiagS", [128, KC, 16, 128], F16,
                                       isOutput=False)
    bkT = nc.declare_dram_parameter("bkT", [128, KC], F32, isOutput=False)
    bvT = nc.declare_dram_parameter("bvT", [128, KC], F32, isOutput=False)
    bgT = nc.declare_dram_parameter("bgT", [128, KC], F32, isOutput=False)
    boutB = nc.declare_dram_parameter("boutB", [128, D], F32, isOutput=False)
    wchan = nc.declare_dram_parameter("wchan", [128, KC, NT], F32, isOutput=False)
    swt = nc.declare_dram_parameter("swt", [128, 2], F32, isOutput=False)
    bo_in = nc.declare_dram_parameter("bo_in", [128, 2], F16, isOutput=False)
    on_in = nc.declare_dram_parameter("on_in", [2, 128], F16, isOutput=False)
    out = nc.declare_dram_parameter("out", [SEQ, D], F32, isOutput=True)

    # ---- internal DRAM (collective staging) ----
    f0_dram = [nc.dram_tensor(f"f0_dram{c}", [128, SEQ], F16)
               for c in range(KC)]
    f0_gath = [nc.dram_tensor(f"f0_gath{c}", [2, 128, SEQ], F16)
               for c in range(KC)]

    with tile.TileContext(nc) as tc:
        with (
            tc.tile_pool(name="psum", bufs=1, space="PSUM") as psp,
            tc.tile_pool(name="const", bufs=1) as constp,
            tc.tile_pool(name="big", bufs=1) as bigp,
        ):
            # ---- constants ----
            bo_t = constp.tile([128, 2], F16)
            nc.sync.dma_start(bo_t[:], bo_in[:])
            on_t = constp.tile([2, 128], F16)
            nc.sync.dma_start(on_t[:], on_in[:])
            wchan_t = constp.tile([128, KC, NT], F32)
            nc.sync.dma_start(wchan_t[:], wchan[:])
            swt_t = constp.tile([128, 2], F32)
            nc.sync.dma_start(swt_t[:], swt[:])
            bk_t = constp.tile([128, KC], F32)
            nc.sync.dma_start(bk_t[:], bkT[:])
            bv_t = constp.tile([128, KC], F32)
            nc.sync.dma_start(bv_t[:], bvT[:])
            bg_t = constp.tile([128, KC], F32)
            nc.sync.dma_start(bg_t[:], bgT[:])
            mask_t = constp.tile([128, 1], F32)
            nc.sync.dma_start(mask_t[:], mask[:])

            gate_t = bigp.tile([128, KC, SEQ], F16, tag="gate")
            field = bigp.tile([128, KC, SEQ], F16, tag="field")

            def psum_tile(tag, bufs):
                return psp.tile([128, 512], F32, tag=tag, bufs=bufs,
                                name=tag)

            # ======== phase B (k/v/f0/gate) + phase C (conv), one scope
            # so conv overlaps the gate matmuls without pool aliasing ====
            with (
                tc.tile_pool(name="p_xm", bufs=1) as p_xm,
                tc.tile_pool(name="p_str", bufs=3) as p_str,
                tc.tile_pool(name="p_ev", bufs=2) as p_ev,
                tc.tile_pool(name="p_cw", bufs=1) as p_cw,
            ):
                xm = p_xm.tile([128, KC, SEQ], F16, tag="xm")
                nc.sync.dma_start(xm[:], xm_in[:])

                def strip_matmuls(sr, evict):
                    """8 stationary chunks x 4 psum banks; evict(rb, psum)."""
                    pss = [psum_tile(f"ps{rb}", 1) for rb in range(4)]
                    for k in range(KC):
                        for rb in range(4):
                            nc.tensor.matmul(
                                pss[rb][:], sr[:, k, :],
                                xm[:, k, rb * 512:(rb + 1) * 512],
                                start=(k == 0), stop=(k == KC - 1))
                    for rb in range(4):
                        evict(rb, pss[rb])

                for c in range(KC):
                    ks = p_str.tile([128, KC, 128], F16, tag="strip")
                    nc.sync.dma_start(ks[:], WkS[:, c, :, :])
                    k2b = p_ev.tile([128, SEQ], F16, tag="k2b", bufs=1)
                    strip_matmuls(
                        ks, lambda rb, ps: nc.scalar.activation(
                            k2b[:, rb * 512:(rb + 1) * 512], ps[:],
                            AF.Square, bias=bk_t[:, c:c + 1]))
                    vs = p_str.tile([128, KC, 128], F16, tag="strip")
                    nc.sync.dma_start(vs[:], WvS[:, c, :, :])
                    vTb = p_ev.tile([128, SEQ], F16, tag="vTb")
                    strip_matmuls(
                        vs, lambda rb, ps: nc.scalar.activation(
                            vTb[:, rb * 512:(rb + 1) * 512], ps[:],
                            AF.Identity, bias=bv_t[:, c:c + 1]))
                    km = p_ev.tile([2, SEQ], F16, tag="km", bufs=1)
                    for sb in range(4):
                        pss = psp.tile([2, 512], F32, tag="km2", bufs=1,
                                       name="km2")
                        nc.tensor.matmul(pss[:], bo_t[:],
                                         k2b[:, sb * 512:(sb + 1) * 512],
                                         start=True, stop=True)
                        nc.scalar.activation(km[:, sb * 512:(sb + 1) * 512],
                                             pss[:], AF.Sqrt)
                    kmagb = p_ev.tile([128, SEQ], F16, tag="kmagb")
                    for sb in range(4):
                        pse = psum_tile("psb", 1)
                        nc.tensor.matmul(pse[:], on_t[:],
                                         km[:, sb * 512:(sb + 1) * 512],
                                         start=True, stop=True)
                        nc.scalar.activation(
                            kmagb[:, sb * 512:(sb + 1) * 512], pse[:],
                            AF.Identity)
                    # f0 = v * k_mag on DVE (all fp16 SBUF -> 4x mode)
                    f0b = p_ev.tile([128, SEQ], F16, tag="f0b")
                    nc.vector.scalar_tensor_tensor(
                        f0b[:], vTb[:], 1.0, kmagb[:],
                        op0=ALU.mult, op1=ALU.mult)
                    nc.sync.dma_start(f0_dram[c][:], f0b[:])
                    nc.gpsimd.collective_compute(
                        "AllGather", ALU.bypass,
                        replica_groups=[[0, 1], [2, 3], [4, 5], [6, 7]],
                        ins=[f0_dram[c][:]], outs=[f0_gath[c][:]])

                # gate = sigmoid(x @ (Wq@Wgate) + b'), stays in SBUF
                for gc in range(KC):
                    gs = p_str.tile([128, KC, 128], F16, tag="strip")
                    nc.sync.dma_start(gs[:], WgS[:, gc, :, :])
                    strip_matmuls(
                        gs, lambda rb, ps: nc.scalar.activation(
                            gate_t[:, gc, rb * 512:(rb + 1) * 512], ps[:],
                            AF.Sigmoid, bias=bg_t[:, gc:gc + 1]))

                # ---- phase C: wavelet FIR + skips (PE+DVE split) ----
                # 16 taps run on PE as diagonal matmuls (host-built diag
                # stationaries) accumulating in 2 rotating PSUM banks with
                # Act eviction; 8 taps chain on DVE; DVE merges + skips.
                exts = []
                for i in range(2):
                    e = p_cw.tile([128, EXT], F16, tag=f"ext{i}")
                    nc.vector.memset(e[:, 0:2048], 0.0)
                    exts.append(e)
                acc = [p_cw.tile([128, CONVN], F16, tag=f"acc{j}", name=f"acc{j}")
                       for j in range(2)]
                tmp = p_cw.tile([128, SEQ], F16, tag="tmp")
                for c in range(KC):
                    ext = exts[c % 2]
                    halo = p_cw.tile([128, SEQ], F16, tag="halo", bufs=2)
                    nc.sync.dma_start(halo[:], f0_gath[c][0, :, :])
                    nc.vector.tensor_scalar_mul(ext[:, 2048:4096], halo[:],
                                                mask_t[:, 0:1])
                    nc.sync.dma_start(ext[:, 4096:EXT], f0_dram[c][:])
                    wd = p_cw.tile([128, 16, 128], F16, tag="wd", bufs=2)
                    nc.sync.dma_start(wd[:], wdiagS[:, c, :, :])
                    # PE taps: per 512-block, accumulate 16 diag matmuls
                    convP = p_cw.tile([128, CONVN], F16, tag="convP", bufs=1)
                    for ob in range(CONVN // 512):
                        psc = psum_tile("cv", 2)
                        for ti, s in enumerate(PE_TAPS):
                            off = 3072 + ob * 512 - s
                            nc.tensor.matmul(psc[:], wd[:, ti, :],
                                             ext[:, off:off + 512],
                                             start=(ti == 0),
                                             stop=(ti == len(PE_TAPS) - 1))
                        nc.scalar.activation(
                            convP[:, ob * 512:(ob + 1) * 512], psc[:],
                            AF.Identity)
                    # DVE taps chain
                    cur = None
                    for ti, s in enumerate(DVE_TAPS):
                        si = SHIFTS.index(s)
                        src_ap = ext[:, 3072 - s:3072 - s + CONVN]
                        w = wchan_t[:, c, si:si + 1]
                        if cur is None:
                            cur = acc[0]
                            nc.vector.tensor_scalar_mul(cur[:], src_ap, w)
                        else:
                            nxt = acc[ti % 2]
                            nc.vector.scalar_tensor_tensor(
                                nxt[:], src_ap, w, cur[:],
                                op0=ALU.mult, op1=ALU.add)
                            cur = nxt
                    # merge PE partial: other acc buffer = convP + cur
                    mrg = acc[(DVE_TAPS.index(DVE_TAPS[-1]) + 1) % 2]
                    nc.vector.scalar_tensor_tensor(
                        mrg[:], convP[:], 1.0, cur[:],
                        op0=ALU.mult, op1=ALU.add)
                    # skips: field[n] = conv[n] + sw0*conv[n-512] + sw1*conv[n-1024]
                    nc.vector.scalar_tensor_tensor(
                        tmp[:], mrg[:, 512:512 + SEQ], swt_t[:, 0:1],
                        mrg[:, 1024:1024 + SEQ],
                        op0=ALU.mult, op1=ALU.add)
                    nc.vector.scalar_tensor_tensor(
                        field[:, c, :], mrg[:, 0:SEQ], swt_t[:, 1:2],
                        tmp[:],
                        op0=ALU.mult, op1=ALU.add)

            # ================= phase E: coupling + gate =================
            with (
                tc.tile_pool(name="p_mc", bufs=1) as p_mc,
                tc.tile_pool(name="p_pg", bufs=1) as p_pg,
                tc.tile_pool(name="p_ev2", bufs=3) as p_ev2,
                tc.tile_pool(name="p_wo", bufs=1) as p_wo,
                tc.tile_pool(name="p_fw", bufs=3) as p_fw,
            ):
                mc_all = p_mc.tile([128, KC, KC, 128], F16, tag="mc")
                nc.sync.dma_start(mc_all[:], McS[:])
                wo_all = p_wo.tile([128, KC, D], F16, tag="wo")
                nc.sync.dma_start(wo_all[:], WoT[:])
                bout_t = p_wo.tile([128, D], F32, tag="bout")
                nc.sync.dma_start(bout_t[:], boutB[:])
                pg = p_pg.tile([128, KC, SEQ], F16, tag="pg")
                for co in range(KC):
                    pss = [psum_tile(f"ps{sb}", 1) for sb in range(4)]
                    for ci in range(KC):
                        for sb in range(4):
                            nc.tensor.matmul(
                                pss[sb][:], mc_all[:, co, ci, :],
                                field[:, ci, sb * 512:(sb + 1) * 512],
                                start=(ci == 0), stop=(ci == KC - 1))
                    for sb in range(4):
                        cpl = p_ev2.tile([128, 512], F16, tag="cpl")
                        nc.scalar.activation(cpl[:], pss[sb][:], AF.Identity)
                        nc.vector.scalar_tensor_tensor(
                            pg[:, co, sb * 512:(sb + 1) * 512],
                            gate_t[:, co, sb * 512:(sb + 1) * 512], 1.0,
                            cpl[:], op0=ALU.mult, op1=ALU.mult)

                # ---- phase F: out = pg @ Wout + bout ----
                for st in range(SEQ // 128):
                    pso = [psum_tile(f"ps{cb}", 1) for cb in range(2)]
                    for k in range(KC):
                        for cb in range(2):
                            nc.tensor.matmul(
                                pso[cb][:],
                                pg[:, k, st * 128:(st + 1) * 128],
                                wo_all[:, k, cb * 512:(cb + 1) * 512],
                                start=(k == 0), stop=(k == KC - 1))
                    outb = p_fw.tile([128, D], F32, tag="outb")
                    for cb in range(2):
                        nc.vector.tensor_add(
                            outb[:, cb * 512:(cb + 1) * 512], pso[cb][:],
                            bout_t[:, cb * 512:(cb + 1) * 512])
                    nc.sync.dma_start(out[st * 128:(st + 1) * 128, :],
                                      outb[:])

    nc.compile()
    _PROGRAM_CACHE["p"] = nc
    return nc


def _softmax(a, axis):
    a = a - a.max(axis=axis, keepdims=True)
    e = np.exp(a)
    return e / e.sum(axis=axis, keepdims=True)


def _host_prep(inputs):
    """Build per-core and replicated input tensors from full inputs."""
    x = np.asarray(inputs["x"], np.float32)
    Wqkv = np.asarray(inputs["Wqkv"], np.float32)
    bqkv = np.asarray(inputs["bqkv"], np.float32)
    Wout = np.asarray(inputs["Wout"], np.float32)
    bout = np.asarray(inputs["bout"], np.float32)
    Wgate = np.asarray(inputs["Wgate"], np.float32)
    bgate = np.asarray(inputs["bgate"], np.float32)
    scale_gain = np.asarray(inputs["scale_gain"], np.float64)
    skip_w = np.asarray(inputs["skip_w"], np.float64)
    coupling = np.asarray(inputs["coupling"], np.float64)

    gains = _softmax(scale_gain, axis=0)              # [11, H]
    sw = 1.0 / (1.0 + np.exp(-skip_w))                # [2]
    coup = _softmax(coupling, axis=-1)                # [H, H]

    sidx = {s: i for i, s in enumerate(SHIFTS)}
    wtab = np.zeros((NT, H), np.float64)
    for j in range(N_SCALES):
        d = 1 << j
        for t in range(4):
            wtab[sidx[(3 - t) * d]] += D4[t] * gains[j]
    ch = np.arange(D)
    wchan = np.zeros((128, KC, NT), np.float32)
    for c in range(KC):
        heads = (ch[c * 128:(c + 1) * 128] // HD)
        wchan[:, c, :] = wtab[:, heads].T.astype(np.float32)

    Mc = np.zeros((D, D), np.float32)
    idx = np.arange(HD)
    for i in range(H):
        for j in range(H):
            Mc[j * HD + idx, i * HD + idx] = coup[i, j]

    # fold the q projection into the gate: gate = sigmoid(x @ (Wq@Wgate) + b')
    Wq = Wqkv[:, :D].astype(np.float64)
    Wqg = (Wq @ Wgate.astype(np.float64)).astype(np.float32)
    bg_f = (bqkv[:D].astype(np.float64) @ Wgate.astype(np.float64)
            + bgate.astype(np.float64)).astype(np.float32)

    def strips(W):
        """[D, D] weight -> [128, KC(strip), KC(contract), 128] fp16."""
        # W[kc*128+p, s*128+j] -> out[p, s, kc, j]
        t = W.reshape(KC, 128, KC, 128)               # [kc, p, s, j]
        return np.ascontiguousarray(
            t.transpose(1, 2, 0, 3).astype(np.float16))

    WkS = strips(Wqkv[:, D:2 * D])
    WvS = strips(Wqkv[:, 2 * D:3 * D])
    WgS = strips(Wqg)
    McS = strips(Mc)
    # Wout moving layout: [p, k, m] = Wout[k*128+p, m]
    WoT = np.ascontiguousarray(
        Wout.reshape(KC, 128, D).transpose(1, 0, 2).astype(np.float16))

    bkT = bqkv[D:2 * D].reshape(KC, 128).T.copy()     # [128, KC]
    bvT = bqkv[2 * D:3 * D].reshape(KC, 128).T.copy()
    bgT = bg_f.reshape(KC, 128).T.copy()
    boutB = np.broadcast_to(bout, (128, D)).copy()
    swt = np.broadcast_to(sw.astype(np.float32), (128, 2)).copy()
    bo = np.zeros((128, 2), np.float16)
    bo[0:64, 0] = 1.0
    bo[64:128, 1] = 1.0
    on = np.zeros((2, 128), np.float16)
    on[0, 0:64] = 1.0
    on[1, 64:128] = 1.0

    # diagonal stationaries for the 16 PE conv taps
    wdiag = np.zeros((128, KC, 16, 128), np.float16)
    for c in range(KC):
        for ti, s in enumerate(PE_TAPS):
            si = SHIFTS.index(s)
            np.fill_diagonal(wdiag[:, c, ti, :], wchan[:, c, si])

    shared = dict(WkS=WkS, WvS=WvS, WgS=WgS, McS=McS, WoT=WoT, wdiagS=wdiag,
                  bkT=bkT, bvT=bvT, bgT=bgT, boutB=boutB, wchan=wchan,
                  swt=swt, bo_in=bo, on_in=on)
    in_maps = []
    for c in range(NCORES):
        b, half = c // 2, c % 2
        g0 = half * SEQ
        # xm[p, kc, n] = x[b, g0+n, kc*128+p]
        xc = x[b, g0:g0 + SEQ, :].reshape(SEQ, KC, 128)
        xm = np.ascontiguousarray(
            xc.transpose(2, 1, 0).astype(np.float16))
        m = np.full((128, 1), float(half), np.float32)
        in_maps.append(dict(xm_in=xm, mask=m, **shared))
    return in_maps


def run_cores(inputs, debug_outputs=False, trace=False):
    nc = _build_program()
    in_maps = _host_prep(inputs)
    res = run_bass_kernel_spmd(nc, in_maps, list(range(NCORES)), trace=trace)
    return res


def kernel(**inputs) -> np.ndarray:
    res = run_cores(inputs)
    out = np.empty((B, N, D), np.float32)
    for c in range(NCORES):
        b, half = c // 2, c % 2
        out[b, half * SEQ:(half + 1) * SEQ, :] = res.results[c]["out"]
    return out


# revision 9
# speedup vs baseline: 1.6389x; 1.1240x over previous
"""Trainium2 Bass kernel for CausalWaveletFieldAttention (v2, fp16 datapath).

Shapes (hardcoded): x [B=4, N=4096, D=1024], H=16 heads, HD=64.
Sharding over 8 cores: core c handles (batch b = c//2, half = c%2), i.e.
2048 contiguous sequence rows of one batch.

Per-core pipeline, all in transposed [channel, seq] layout, fp16 SBUF
datapath (output tolerance is 2e-2; fp16 keeps us ~1e-3):
  1. k/v projections on PE (fp16 matmuls, k-outer loop so each stationary
     strip is reused across 4 PSUM banks), Act evicts with Square/Identity
  2. k_mag via PE block-ones reduce + Act Sqrt + PE broadcast,
     f0 = v * k_mag fused on DVE (4x fp16 mode)
  3. pairwise fp16 AllGather of f0 halves (causal-conv history)
  4. gate = sigmoid(x @ (Wq@Wgate) + b') on PE, Act Sigmoid evict,
     kept in SBUF (no DRAM round trip)
  5. 24-tap dilated wavelet FIR entirely on DVE as scalar_tensor_tensor
     chains over [128, 3072] fp16 tiles (4x packed mode, ~800ns/tap),
     extended 1024 cols back so the d=512/1024 skip taps apply locally;
     overlaps the gate matmuls on PE
  6. head coupling as dense [1024,1024] fp16 matmul on PE; Act evicts,
     DVE multiplies by the SBUF-resident gate (4x)
  7. out = (field*gate) @ Wout + bout with pg strips as PE stationaries
     straight from SBUF (no transpose round trip)
"""

import numpy as np
import ml_dtypes

import concourse.bass as bass
import concourse.mybir as mybir
import concourse.tile as tile
from concourse import bacc
from concourse.bass_utils import run_bass_kernel_spmd

F32 = mybir.dt.float32
F16 = mybir.dt.float16
BF16 = mybir.dt.bfloat16
AF = mybir.ActivationFunctionType
ALU = mybir.AluOpType

B, N, D, H, HD = 4, 4096, 1024, 16, 64
NCORES = 8
SEQ = N // 2          # 2048 rows per core
KC = D // 128         # 8 contraction chunks
CONVN = SEQ + 1024    # 3072 conv outputs (1024 extra for skip taps)
EXT = SEQ + 4096      # 6144 extended f0 buffer
D4 = [0.4829629131445341, 0.8365163037378079, 0.2241438680420134, -0.1294095225512604]
N_SCALES = 11
SPARSE_DILATIONS = (512, 1024)
SHIFTS = [0, 1, 2, 3, 4, 6, 8, 12, 16, 24, 32, 48, 64, 96, 128, 192, 256,
          384, 512, 768, 1024, 1536, 2048, 3072]
NT = len(SHIFTS)      # 24 taps
DVE_TAPS = [512, 768, 1024, 1536, 2048, 3072]
PE_TAPS = [s for s in SHIFTS if s not in DVE_TAPS]   # 18 taps

_PROGRAM_CACHE = {}


def _build_program():
    if "p" in _PROGRAM_CACHE:
        return _PROGRAM_CACHE["p"]

    nc = bacc.Bacc("TRN2", target_bir_lowering=False, debug=False,
                   num_devices=NCORES)

    # ---- parameters (per-core); weights pre-packed partition-major ----
    xm_in = nc.declare_dram_parameter("xm_in", [128, KC, SEQ], F16, isOutput=False)
    mask = nc.declare_dram_parameter("mask", [128, 1], F32, isOutput=False)
    WkS = nc.declare_dram_parameter("WkS", [128, KC, KC, 128], F16, isOutput=False)
    WvS = nc.declare_dram_parameter("WvS", [128, KC, KC, 128], F16, isOutput=False)
    WgS = nc.declare_dram_parameter("WgS", [128, KC, KC, 128], F16, isOutput=False)
    McS = nc.declare_dram_parameter("McS", [128, KC, KC, 128], F16, isOutput=False)
    WoT = nc.declare_dram_parameter("WoT", [128, KC, D], F16, isOutput=False)
    wdiagS = nc.declare_dram_parameter("wdiagS", [128, KC, 18, 128], F16,
                                       isOutput=False)
    bkT = nc.declare_dram_parameter("bkT", [128, KC], F32, isOutput=False)
    bvT = nc.declare_dram_parameter("bvT", [128, KC], F32, isOutput=False)
    bgT = nc.declare_dram_parameter("bgT", [128, KC], F32, isOutput=False)
    boutB = nc.declare_dram_parameter("boutB", [128, D], F32, isOutput=False)
    wchan = nc.declare_dram_parameter("wchan", [128, KC, NT], F32, isOutput=False)
    swt = nc.declare_dram_parameter("swt", [128, 2], F32, isOutput=False)
    bo_in = nc.declare_dram_parameter("bo_in", [128, 2], F16, isOutput=False)
    on_in = nc.declare_dram_parameter("on_in", [2, 128], F16, isOutput=False)
    out = nc.declare_dram_parameter("out", [SEQ, D], F32, isOutput=True)

    # ---- internal DRAM (collective staging) ----
    f0_dram = [nc.dram_tensor(f"f0_dram{c}", [128, SEQ], F16)
               for c in range(KC)]
    f0_gath = [nc.dram_tensor(f"f0_gath{c}", [2, 128, SEQ], F16)
               for c in range(KC)]

    with tile.TileContext(nc) as tc:
        with (
            tc.tile_pool(name="psum", bufs=1, space="PSUM") as psp,
            tc.tile_pool(name="const", bufs=1) as constp,
            tc.tile_pool(name="big", bufs=1) as bigp,
        ):
            # ---- constants ----
            bo_t = constp.tile([128, 2], F16)
            nc.sync.dma_start(bo_t[:], bo_in[:])
            on_t = constp.tile([2, 128], F16)
            nc.sync.dma_start(on_t[:], on_in[:])
            wchan_t = constp.tile([128, KC, NT], F32)
            nc.sync.dma_start(wchan_t[:], wchan[:])
            swt_t = constp.tile([128, 2], F32)
            nc.sync.dma_start(swt_t[:], swt[:])
            bk_t = constp.tile([128, KC], F32)
            nc.sync.dma_start(bk_t[:], bkT[:])
            bv_t = constp.tile([128, KC], F32)
            nc.sync.dma_start(bv_t[:], bvT[:])
            bg_t = constp.tile([128, KC], F32)
            nc.sync.dma_start(bg_t[:], bgT[:])
            mask_t = constp.tile([128, 1], F32)
            nc.sync.dma_start(mask_t[:], mask[:])

            gate_t = bigp.tile([128, KC, SEQ], F16, tag="gate")
            field = bigp.tile([128, KC, SEQ], F16, tag="field")

            def psum_tile(tag, bufs):
                return psp.tile([128, 512], F32, tag=tag, bufs=bufs,
                                name=tag)

            # ======== phase B (k/v/f0/gate) + phase C (conv), one scope
            # so conv overlaps the gate matmuls without pool aliasing ====
            with (
                tc.tile_pool(name="p_xm", bufs=1) as p_xm,
                tc.tile_pool(name="p_str", bufs=3) as p_str,
                tc.tile_pool(name="p_ev", bufs=2) as p_ev,
                tc.tile_pool(name="p_cw", bufs=1) as p_cw,
            ):
                xm = p_xm.tile([128, KC, SEQ], F16, tag="xm")
                nc.sync.dma_start(xm[:], xm_in[:])

                def strip_matmuls(sr, evict):
                    """8 stationary chunks x 4 psum banks; evict(rb, psum)."""
                    pss = [psum_tile(f"ps{rb}", 1) for rb in range(4)]
                    for k in range(KC):
                        for rb in range(4):
                            nc.tensor.matmul(
                                pss[rb][:], sr[:, k, :],
                                xm[:, k, rb * 512:(rb + 1) * 512],
                                start=(k == 0), stop=(k == KC - 1))
                    for rb in range(4):
                        evict(rb, pss[rb])

                for c in range(KC):
                    ks = p_str.tile([128, KC, 128], F16, tag="strip")
                    nc.sync.dma_start(ks[:], WkS[:, c, :, :])
                    k2b = p_ev.tile([128, SEQ], F16, tag="k2b", bufs=1)
                    strip_matmuls(
                        ks, lambda rb, ps: nc.scalar.activation(
                            k2b[:, rb * 512:(rb + 1) * 512], ps[:],
                            AF.Square, bias=bk_t[:, c:c + 1]))
                    vs = p_str.tile([128, KC, 128], F16, tag="strip")
                    nc.sync.dma_start(vs[:], WvS[:, c, :, :])
                    vTb = p_ev.tile([128, SEQ], F16, tag="vTb")
                    strip_matmuls(
                        vs, lambda rb, ps: nc.scalar.activation(
                            vTb[:, rb * 512:(rb + 1) * 512], ps[:],
                            AF.Identity, bias=bv_t[:, c:c + 1]))
                    km = p_ev.tile([2, SEQ], F16, tag="km", bufs=1)
                    for sb in range(4):
                        pss = psp.tile([2, 512], F32, tag="km2", bufs=1,
                                       name="km2")
                        nc.tensor.matmul(pss[:], bo_t[:],
                                         k2b[:, sb * 512:(sb + 1) * 512],
                                         start=True, stop=True)
                        nc.scalar.activation(km[:, sb * 512:(sb + 1) * 512],
                                             pss[:], AF.Sqrt)
                    kmagb = p_ev.tile([128, SEQ], F16, tag="kmagb")
                    for sb in range(4):
                        pse = psum_tile("psb", 1)
                        nc.tensor.matmul(pse[:], on_t[:],
                                         km[:, sb * 512:(sb + 1) * 512],
                                         start=True, stop=True)
                        nc.scalar.activation(
                            kmagb[:, sb * 512:(sb + 1) * 512], pse[:],
                            AF.Identity)
                    # f0 = v * k_mag on DVE (all fp16 SBUF -> 4x mode)
                    f0b = p_ev.tile([128, SEQ], F16, tag="f0b")
                    nc.vector.scalar_tensor_tensor(
                        f0b[:], vTb[:], 1.0, kmagb[:],
                        op0=ALU.mult, op1=ALU.mult)
                    nc.sync.dma_start(f0_dram[c][:], f0b[:])
                    nc.gpsimd.collective_compute(
                        "AllGather", ALU.bypass,
                        replica_groups=[[0, 1], [2, 3], [4, 5], [6, 7]],
                        ins=[f0_dram[c][:]], outs=[f0_gath[c][:]])

                # ---- phase C: wavelet FIR + skips (PE+DVE split) ----
                # 16 taps run on PE as diagonal matmuls (host-built diag
                # stationaries) accumulating in 2 rotating PSUM banks with
                # Act eviction; 8 taps chain on DVE; DVE merges + skips.
                exts = []
                for i in range(2):
                    e = p_cw.tile([128, EXT], F16, tag=f"ext{i}")
                    nc.vector.memset(e[:, 0:2048], 0.0)
                    exts.append(e)
                acc = [p_cw.tile([128, CONVN], F16, tag=f"acc{j}", name=f"acc{j}")
                       for j in range(2)]
                tmp = p_cw.tile([128, SEQ], F16, tag="tmp")
                for c in range(KC):
                    ext = exts[c % 2]
                    halo = p_cw.tile([128, SEQ], F16, tag="halo", bufs=2)
                    nc.sync.dma_start(halo[:], f0_gath[c][0, :, :])
                    nc.vector.tensor_scalar_mul(ext[:, 2048:4096], halo[:],
                                                mask_t[:, 0:1])
                    nc.sync.dma_start(ext[:, 4096:EXT], f0_dram[c][:])
                    wd = p_cw.tile([128, 18, 128], F16, tag="wd", bufs=2)
                    nc.sync.dma_start(wd[:], wdiagS[:, c, :, :])
                    # PE taps: per 512-block, accumulate 16 diag matmuls
                    convP = p_cw.tile([128, CONVN], F16, tag="convP", bufs=1)
                    for ob in range(CONVN // 512):
                        psc = psum_tile("cv", 2)
                        for ti, s in enumerate(PE_TAPS):
                            off = 3072 + ob * 512 - s
                            nc.tensor.matmul(psc[:], wd[:, ti, :],
                                             ext[:, off:off + 512],
                                             start=(ti == 0),
                                             stop=(ti == len(PE_TAPS) - 1))
                        nc.scalar.activation(
                            convP[:, ob * 512:(ob + 1) * 512], psc[:],
                            AF.Identity)
                    # DVE taps chain
                    cur = None
                    for ti, s in enumerate(DVE_TAPS):
                        si = SHIFTS.index(s)
                        src_ap = ext[:, 3072 - s:3072 - s + CONVN]
                        w = wchan_t[:, c, si:si + 1]
                        if cur is None:
                            cur = acc[0]
                            nc.vector.tensor_scalar_mul(cur[:], src_ap, w)
                        else:
                            nxt = acc[ti % 2]
                            nc.vector.scalar_tensor_tensor(
                                nxt[:], src_ap, w, cur[:],
                                op0=ALU.mult, op1=ALU.add)
                            cur = nxt
                    # merge PE partial: other acc buffer = convP + cur
                    mrg = acc[(DVE_TAPS.index(DVE_TAPS[-1]) + 1) % 2]
                    nc.vector.scalar_tensor_tensor(
                        mrg[:], convP[:], 1.0, cur[:],
                        op0=ALU.mult, op1=ALU.add)
                    # skips: field[n] = conv[n] + sw0*conv[n-512] + sw1*conv[n-1024]
                    nc.vector.scalar_tensor_tensor(
                        tmp[:], mrg[:, 512:512 + SEQ], swt_t[:, 0:1],
                        mrg[:, 1024:1024 + SEQ],
                        op0=ALU.mult, op1=ALU.add)
                    nc.vector.scalar_tensor_tensor(
                        field[:, c, :], mrg[:, 0:SEQ], swt_t[:, 1:2],
                        tmp[:],
                        op0=ALU.mult, op1=ALU.add)

                    # gate strip c interleaved so PE stays busy between
                    # conv chunks (gate = sigmoid(x @ (Wq@Wgate) + b'))
                    gs = p_str.tile([128, KC, 128], F16, tag="strip")
                    nc.sync.dma_start(gs[:], WgS[:, c, :, :])
                    strip_matmuls(
                        gs, lambda rb, ps: nc.scalar.activation(
                            gate_t[:, c, rb * 512:(rb + 1) * 512], ps[:],
                            AF.Sigmoid, bias=bg_t[:, c:c + 1]))

            # ================= phase E: coupling + gate =================
            with (
                tc.tile_pool(name="p_mc", bufs=1) as p_mc,
                tc.tile_pool(name="p_pg", bufs=1) as p_pg,
                tc.tile_pool(name="p_ev2", bufs=3) as p_ev2,
                tc.tile_pool(name="p_wo", bufs=1) as p_wo,
                tc.tile_pool(name="p_fw", bufs=3) as p_fw,
            ):
                mc_all = p_mc.tile([128, KC, KC, 128], F16, tag="mc")
                nc.sync.dma_start(mc_all[:], McS[:])
                wo_all = p_wo.tile([128, KC, D], F16, tag="wo")
                nc.sync.dma_start(wo_all[:], WoT[:])
                bout_t = p_wo.tile([128, D], F32, tag="bout")
                nc.sync.dma_start(bout_t[:], boutB[:])
                pg = p_pg.tile([128, KC, SEQ], F16, tag="pg")
                for co in range(KC):
                    pss = [psum_tile(f"ps{sb}", 1) for sb in range(4)]
                    for ci in range(KC):
                        for sb in range(4):
                            nc.tensor.matmul(
                                pss[sb][:], mc_all[:, co, ci, :],
                                field[:, ci, sb * 512:(sb + 1) * 512],
                                start=(ci == 0), stop=(ci == KC - 1))
                    for sb in range(4):
                        cpl = p_ev2.tile([128, 512], F16, tag="cpl")
                        nc.scalar.activation(cpl[:], pss[sb][:], AF.Identity)
                        nc.vector.scalar_tensor_tensor(
                            pg[:, co, sb * 512:(sb + 1) * 512],
                            gate_t[:, co, sb * 512:(sb + 1) * 512], 1.0,
                            cpl[:], op0=ALU.mult, op1=ALU.mult)

                # ---- phase F: out = pg @ Wout + bout ----
                for st in range(SEQ // 128):
                    pso = [psum_tile(f"ps{cb}", 1) for cb in range(2)]
                    for k in range(KC):
                        for cb in range(2):
                            nc.tensor.matmul(
                                pso[cb][:],
                                pg[:, k, st * 128:(st + 1) * 128],
                                wo_all[:, k, cb * 512:(cb + 1) * 512],
                                start=(k == 0), stop=(k == KC - 1))
                    outb = p_fw.tile([128, D], F32, tag="outb")
                    for cb in range(2):
                        nc.vector.tensor_add(
                            outb[:, cb * 512:(cb + 1) * 512], pso[cb][:],
                            bout_t[:, cb * 512:(cb + 1) * 512])
                    nc.sync.dma_start(out[st * 128:(st + 1) * 128, :],
                                      outb[:])

    nc.compile()
    _PROGRAM_CACHE["p"] = nc
    return nc


def _softmax(a, axis):
    a = a - a.max(axis=axis, keepdims=True)
    e = np.exp(a)
    return e / e.sum(axis=axis, keepdims=True)


def _host_prep(inputs):
    """Build per-core and replicated input tensors from full inputs."""
    x = np.asarray(inputs["x"], np.float32)
    Wqkv = np.asarray(inputs["Wqkv"], np.float32)
    bqkv = np.asarray(inputs["bqkv"], np.float32)
    Wout = np.asarray(inputs["Wout"], np.float32)
    bout = np.asarray(inputs["bout"], np.float32)
    Wgate = np.asarray(inputs["Wgate"], np.float32)
    bgate = np.asarray(inputs["bgate"], np.float32)
    scale_gain = np.asarray(inputs["scale_gain"], np.float64)
    skip_w = np.asarray(inputs["skip_w"], np.float64)
    coupling = np.asarray(inputs["coupling"], np.float64)

    gains = _softmax(scale_gain, axis=0)              # [11, H]
    sw = 1.0 / (1.0 + np.exp(-skip_w))                # [2]
    coup = _softmax(coupling, axis=-1)                # [H, H]

    sidx = {s: i for i, s in enumerate(SHIFTS)}
    wtab = np.zeros((NT, H), np.float64)
    for j in range(N_SCALES):
        d = 1 << j
        for t in range(4):
            wtab[sidx[(3 - t) * d]] += D4[t] * gains[j]
    ch = np.arange(D)
    wchan = np.zeros((128, KC, NT), np.float32)
    for c in range(KC):
        heads = (ch[c * 128:(c + 1) * 128] // HD)
        wchan[:, c, :] = wtab[:, heads].T.astype(np.float32)

    Mc = np.zeros((D, D), np.float32)
    idx = np.arange(HD)
    for i in range(H):
        for j in range(H):
            Mc[j * HD + idx, i * HD + idx] = coup[i, j]

    # fold the q projection into the gate: gate = sigmoid(x @ (Wq@Wgate) + b')
    Wq = Wqkv[:, :D].astype(np.float64)
    Wqg = (Wq @ Wgate.astype(np.float64)).astype(np.float32)
    bg_f = (bqkv[:D].astype(np.float64) @ Wgate.astype(np.float64)
            + bgate.astype(np.float64)).astype(np.float32)

    def strips(W):
        """[D, D] weight -> [128, KC(strip), KC(contract), 128] fp16."""
        # W[kc*128+p, s*128+j] -> out[p, s, kc, j]
        t = W.reshape(KC, 128, KC, 128)               # [kc, p, s, j]
        return np.ascontiguousarray(
            t.transpose(1, 2, 0, 3).astype(np.float16))

    WkS = strips(Wqkv[:, D:2 * D])
    WvS = strips(Wqkv[:, 2 * D:3 * D])
    WgS = strips(Wqg)
    McS = strips(Mc)
    # Wout moving layout: [p, k, m] = Wout[k*128+p, m]
    WoT = np.ascontiguousarray(
        Wout.reshape(KC, 128, D).transpose(1, 0, 2).astype(np.float16))

    bkT = bqkv[D:2 * D].reshape(KC, 128).T.copy()     # [128, KC]
    bvT = bqkv[2 * D:3 * D].reshape(KC, 128).T.copy()
    bgT = bg_f.reshape(KC, 128).T.copy()
    boutB = np.broadcast_to(bout, (128, D)).copy()
    swt = np.broadcast_to(sw.astype(np.float32), (128, 2)).copy()
    bo = np.zeros((128, 2), np.float16)
    bo[0:64, 0] = 1.0
    bo[64:128, 1] = 1.0
    on = np.zeros((2, 128), np.float16)
    on[0, 0:64] = 1.0
    on[1, 64:128] = 1.0

    # diagonal stationaries for the 16 PE conv taps
    wdiag = np.zeros((128, KC, 18, 128), np.float16)
    for c in range(KC):
        for ti, s in enumerate(PE_TAPS):
            si = SHIFTS.index(s)
            np.fill_diagonal(wdiag[:, c, ti, :], wchan[:, c, si])

    shared = dict(WkS=WkS, WvS=WvS, WgS=WgS, McS=McS, WoT=WoT, wdiagS=wdiag,
                  bkT=bkT, bvT=bvT, bgT=bgT, boutB=boutB, wchan=wchan,
                  swt=swt, bo_in=bo, on_in=on)
    in_maps = []
    for c in range(NCORES):
        b, half = c // 2, c % 2
        g0 = half * SEQ
        # xm[p, kc, n] = x[b, g0+n, kc*128+p]
        xc = x[b, g0:g0 + SEQ, :].reshape(SEQ, KC, 128)
        xm = np.ascontiguousarray(
            xc.transpose(2, 1, 0).astype(np.float16))
        m = np.full((128, 1), float(half), np.float32)
        in_maps.append(dict(xm_in=xm, mask=m, **shared))
    return in_maps


def run_cores(inputs, debug_outputs=False, trace=False):
    nc = _build_program()
    in_maps = _host_prep(inputs)
    res = run_bass_kernel_spmd(nc, in_maps, list(range(NCORES)), trace=trace)
    return res


def kernel(**inputs) -> np.ndarray:
    res = run_cores(inputs)
    out = np.empty((B, N, D), np.float32)
    for c in range(NCORES):
        b, half = c // 2, c % 2
        out[b, half * SEQ:(half + 1) * SEQ, :] = res.results[c]["out"]
    return out


# revision 12
# speedup vs baseline: 1.6901x; 1.0312x over previous
"""Trainium2 Bass kernel for CausalWaveletFieldAttention (v2, fp16 datapath).

Shapes (hardcoded): x [B=4, N=4096, D=1024], H=16 heads, HD=64.
Sharding over 8 cores: core c handles (batch b = c//2, half = c%2), i.e.
2048 contiguous sequence rows of one batch.

Per-core pipeline, all in transposed [channel, seq] layout, fp16 SBUF
datapath (output tolerance is 2e-2; fp16 keeps us ~1e-3):
  1. k/v projections on PE (fp16 matmuls, k-outer loop so each stationary
     strip is reused across 4 PSUM banks), Act evicts with Square/Identity
  2. k_mag via PE block-ones reduce + Act Sqrt + PE broadcast,
     f0 = v * k_mag fused on DVE (4x fp16 mode)
  3. pairwise fp16 AllGather of f0 halves (causal-conv history)
  4. gate = sigmoid(x @ (Wq@Wgate) + b') on PE, Act Sigmoid evict,
     kept in SBUF (no DRAM round trip)
  5. 24-tap dilated wavelet FIR entirely on DVE as scalar_tensor_tensor
     chains over [128, 3072] fp16 tiles (4x packed mode, ~800ns/tap),
     extended 1024 cols back so the d=512/1024 skip taps apply locally;
     overlaps the gate matmuls on PE
  6. head coupling as dense [1024,1024] fp16 matmul on PE; Act evicts,
     DVE multiplies by the SBUF-resident gate (4x)
  7. out = (field*gate) @ Wout + bout with pg strips as PE stationaries
     straight from SBUF (no transpose round trip)
"""

import numpy as np

import concourse.bass as bass
import concourse.mybir as mybir
import concourse.tile as tile
from concourse import bacc
from concourse.bass_utils import run_bass_kernel_spmd

F32 = mybir.dt.float32
F16 = mybir.dt.float16
BF16 = mybir.dt.bfloat16
AF = mybir.ActivationFunctionType
ALU = mybir.AluOpType

B, N, D, H, HD = 4, 4096, 1024, 16, 64
NCORES = 8
SEQ = N // 2          # 2048 rows per core
KC = D // 128         # 8 contraction chunks
CONVN = SEQ + 1024    # 3072 conv outputs (1024 extra for skip taps)
EXT = SEQ + 4096      # 6144 extended f0 buffer
D4 = [0.4829629131445341, 0.8365163037378079, 0.2241438680420134, -0.1294095225512604]
N_SCALES = 11
SPARSE_DILATIONS = (512, 1024)
SHIFTS = [0, 1, 2, 3, 4, 6, 8, 12, 16, 24, 32, 48, 64, 96, 128, 192, 256,
          384, 512, 768, 1024, 1536, 2048, 3072]
NT = len(SHIFTS)      # 24 taps
DVE_TAPS = [512, 768, 1024, 1536, 2048, 3072]
PE_TAPS = [s for s in SHIFTS if s not in DVE_TAPS]   # 18 taps

_PROGRAM_CACHE = {}


def _build_program():
    if "p" in _PROGRAM_CACHE:
        return _PROGRAM_CACHE["p"]

    nc = bacc.Bacc("TRN2", target_bir_lowering=False, debug=False,
                   num_devices=NCORES)

    # ---- parameters (per-core); weights pre-packed partition-major ----
    xm_in = nc.declare_dram_parameter("xm_in", [128, KC, SEQ], F16, isOutput=False)
    mask = nc.declare_dram_parameter("mask", [128, 1], F32, isOutput=False)
    WkS = nc.declare_dram_parameter("WkS", [128, KC, KC, 128], F16, isOutput=False)
    WvS = nc.declare_dram_parameter("WvS", [128, KC, KC, 128], F16, isOutput=False)
    WgS = nc.declare_dram_parameter("WgS", [128, KC, KC, 128], F16, isOutput=False)
    mcB_in = nc.declare_dram_parameter("mcB_in", [128, 128], F16, isOutput=False)
    WoT = nc.declare_dram_parameter("WoT", [128, KC, D], F16, isOutput=False)
    wdiagS = nc.declare_dram_parameter("wdiagS", [128, KC, 18, 128], F16,
                                       isOutput=False)
    bkT = nc.declare_dram_parameter("bkT", [128, KC], F32, isOutput=False)
    bvT = nc.declare_dram_parameter("bvT", [128, KC], F32, isOutput=False)
    bgT = nc.declare_dram_parameter("bgT", [128, KC], F32, isOutput=False)
    boutB = nc.declare_dram_parameter("boutB", [128, D], F32, isOutput=False)
    wchan = nc.declare_dram_parameter("wchan", [128, KC, NT], F32, isOutput=False)
    swt = nc.declare_dram_parameter("swt", [128, 2], F32, isOutput=False)
    boS = nc.declare_dram_parameter("boS", [128, KC, 16], F16, isOutput=False)
    on_in = nc.declare_dram_parameter("on_in", [16, 128], F16, isOutput=False)
    out = nc.declare_dram_parameter("out", [SEQ, D], F32, isOutput=True)

    # ---- internal DRAM (collective staging) ----
    f0_dram = [nc.dram_tensor(f"f0_dram{c}", [128, SEQ], F16)
               for c in range(KC)]
    f0_gath = [nc.dram_tensor(f"f0_gath{c}", [2, 128, SEQ], F16)
               for c in range(KC)]

    with tile.TileContext(nc) as tc:
        with (
            tc.tile_pool(name="psum", bufs=1, space="PSUM") as psp,
            tc.tile_pool(name="const", bufs=1) as constp,
            tc.tile_pool(name="big", bufs=1) as bigp,
        ):
            # ---- constants ----
            bo_t = constp.tile([128, KC, 16], F16)
            nc.sync.dma_start(bo_t[:], boS[:])
            on_t = constp.tile([16, 128], F16)
            nc.sync.dma_start(on_t[:], on_in[:])
            mcB_t = constp.tile([128, 128], F16)
            nc.sync.dma_start(mcB_t[:], mcB_in[:])
            wchan_t = constp.tile([128, KC, NT], F32)
            nc.sync.dma_start(wchan_t[:], wchan[:])
            swt_t = constp.tile([128, 2], F32)
            nc.sync.dma_start(swt_t[:], swt[:])
            bk_t = constp.tile([128, KC], F32)
            nc.sync.dma_start(bk_t[:], bkT[:])
            bv_t = constp.tile([128, KC], F32)
            nc.sync.dma_start(bv_t[:], bvT[:])
            bg_t = constp.tile([128, KC], F32)
            nc.sync.dma_start(bg_t[:], bgT[:])
            mask_t = constp.tile([128, 1], F32)
            nc.sync.dma_start(mask_t[:], mask[:])

            gate_t = bigp.tile([128, KC, SEQ], F16, tag="gate")
            field = bigp.tile([128, KC, SEQ], F16, tag="field")

            def psum_tile(tag, bufs):
                return psp.tile([128, 512], F32, tag=tag, bufs=bufs,
                                name=tag)

            # ======== phase B (k/v/f0/gate) + phase C (conv), one scope
            # so conv overlaps the gate matmuls without pool aliasing ====
            with (
                tc.tile_pool(name="p_xm", bufs=1) as p_xm,
                tc.tile_pool(name="p_str", bufs=3) as p_str,
                tc.tile_pool(name="p_ev", bufs=2) as p_ev,
                tc.tile_pool(name="p_cw", bufs=1) as p_cw,
            ):
                xm = p_xm.tile([128, KC, SEQ], F16, tag="xm")
                for k in range(KC):
                    nc.sync.dma_start(xm[:, k, :], xm_in[:, k, :])

                def strip_matmuls(sr, evict):
                    """8 stationary chunks x 4 psum banks; evict(rb, psum)."""
                    pss = [psum_tile(f"ps{rb}", 1) for rb in range(4)]
                    for k in range(KC):
                        for rb in range(4):
                            nc.tensor.matmul(
                                pss[rb][:], sr[:, k, :],
                                xm[:, k, rb * 512:(rb + 1) * 512],
                                start=(k == 0), stop=(k == KC - 1))
                    for rb in range(4):
                        evict(rb, pss[rb])

                # k strips (h-major); km^2 accumulates across chunks in
                # 4 PSUM tiles [16,512] (rows 2c,2c+1 per chunk via boS)
                kmsb = [psp.tile([16, 512], F32, tag=t, bufs=b, name=t)
                        for t, b in (("cv", 2), ("cv", 2), ("psb", 1), ("km2", 1))]
                for c in range(KC):
                    ks = p_str.tile([128, KC, 128], F16, tag="strip")
                    nc.sync.dma_start(ks[:], WkS[:, c, :, :])
                    k2b = p_ev.tile([128, SEQ], F16, tag="k2b", bufs=1)
                    strip_matmuls(
                        ks, lambda rb, ps: nc.scalar.activation(
                            k2b[:, rb * 512:(rb + 1) * 512], ps[:],
                            AF.Square, bias=bk_t[:, c:c + 1]))
                    for sb in range(4):
                        nc.tensor.matmul(kmsb[sb][:], bo_t[:, c, :],
                                         k2b[:, sb * 512:(sb + 1) * 512],
                                         start=(c == 0), stop=(c == KC - 1),
                                         skip_group_check=True)
                km_all = p_ev.tile([16, SEQ], F16, tag="km", bufs=1)
                for sb in range(4):
                    nc.scalar.activation(km_all[:, sb * 512:(sb + 1) * 512],
                                         kmsb[sb][:], AF.Sqrt)
                # k_mag broadcast: identical for every d-major chunk
                kmagb = p_ev.tile([128, SEQ], F16, tag="kmagb", bufs=1)
                for sb in range(4):
                    pse = psum_tile("psb", 1)
                    nc.tensor.matmul(pse[:], on_t[:],
                                     km_all[:, sb * 512:(sb + 1) * 512],
                                     start=True, stop=True)
                    nc.scalar.activation(
                        kmagb[:, sb * 512:(sb + 1) * 512], pse[:],
                        AF.Identity)
                # v strips (d-major) -> f0, store, pairwise gather
                for c in range(KC):
                    vs = p_str.tile([128, KC, 128], F16, tag="strip")
                    nc.sync.dma_start(vs[:], WvS[:, c, :, :])
                    vTb = p_ev.tile([128, SEQ], F16, tag="vTb")
                    strip_matmuls(
                        vs, lambda rb, ps: nc.scalar.activation(
                            vTb[:, rb * 512:(rb + 1) * 512], ps[:],
                            AF.Identity, bias=bv_t[:, c:c + 1]))
                    f0b = p_ev.tile([128, SEQ], F16, tag="f0b")
                    nc.vector.scalar_tensor_tensor(
                        f0b[:], vTb[:], 1.0, kmagb[:],
                        op0=ALU.mult, op1=ALU.mult)
                    nc.sync.dma_start(f0_dram[c][:], f0b[:])
                    nc.gpsimd.collective_compute(
                        "AllGather", ALU.bypass,
                        replica_groups=[[0, 1], [2, 3], [4, 5], [6, 7]],
                        ins=[f0_dram[c][:]], outs=[f0_gath[c][:]])

                # ---- phase C: wavelet FIR + skips (PE+DVE split) ----
                # 16 taps run on PE as diagonal matmuls (host-built diag
                # stationaries) accumulating in 2 rotating PSUM banks with
                # Act eviction; 8 taps chain on DVE; DVE merges + skips.
                exts = []
                for i in range(2):
                    e = p_cw.tile([128, EXT], F16, tag=f"ext{i}")
                    nc.vector.memset(e[:, 0:2048], 0.0)
                    exts.append(e)
                acc = [p_cw.tile([128, CONVN], F16, tag=f"acc{j}", name=f"acc{j}")
                       for j in range(2)]
                tmp = p_cw.tile([128, SEQ], F16, tag="tmp")
                for c in range(KC):
                    ext = exts[c % 2]
                    halo = p_cw.tile([128, SEQ], F16, tag="halo", bufs=2)
                    nc.sync.dma_start(halo[:], f0_gath[c][0, :, :])
                    nc.vector.tensor_scalar_mul(ext[:, 2048:4096], halo[:],
                                                mask_t[:, 0:1])
                    nc.sync.dma_start(ext[:, 4096:EXT], f0_dram[c][:])
                    wd = p_cw.tile([128, 18, 128], F16, tag="wd", bufs=2)
                    nc.sync.dma_start(wd[:], wdiagS[:, c, :, :])
                    # PE taps: per 512-block, accumulate 16 diag matmuls
                    convP = p_cw.tile([128, CONVN], F16, tag="convP", bufs=1)
                    for ob in range(CONVN // 512):
                        psc = psum_tile("cv", 2)
                        for ti, s in enumerate(PE_TAPS):
                            off = 3072 + ob * 512 - s
                            nc.tensor.matmul(psc[:], wd[:, ti, :],
                                             ext[:, off:off + 512],
                                             start=(ti == 0),
                                             stop=(ti == len(PE_TAPS) - 1))
                        nc.scalar.activation(
                            convP[:, ob * 512:(ob + 1) * 512], psc[:],
                            AF.Identity)
                    # DVE taps chain
                    cur = None
                    for ti, s in enumerate(DVE_TAPS):
                        si = SHIFTS.index(s)
                        src_ap = ext[:, 3072 - s:3072 - s + CONVN]
                        w = wchan_t[:, c, si:si + 1]
                        if cur is None:
                            cur = acc[0]
                            nc.vector.tensor_scalar_mul(cur[:], src_ap, w)
                        else:
                            nxt = acc[ti % 2]
                            nc.vector.scalar_tensor_tensor(
                                nxt[:], src_ap, w, cur[:],
                                op0=ALU.mult, op1=ALU.add)
                            cur = nxt
                    # merge PE partial: other acc buffer = convP + cur
                    mrg = acc[(DVE_TAPS.index(DVE_TAPS[-1]) + 1) % 2]
                    nc.vector.scalar_tensor_tensor(
                        mrg[:], convP[:], 1.0, cur[:],
                        op0=ALU.mult, op1=ALU.add)
                    # skips: field[n] = conv[n] + sw0*conv[n-512] + sw1*conv[n-1024]
                    nc.vector.scalar_tensor_tensor(
                        tmp[:], mrg[:, 512:512 + SEQ], swt_t[:, 0:1],
                        mrg[:, 1024:1024 + SEQ],
                        op0=ALU.mult, op1=ALU.add)
                    nc.vector.scalar_tensor_tensor(
                        field[:, c, :], mrg[:, 0:SEQ], swt_t[:, 1:2],
                        tmp[:],
                        op0=ALU.mult, op1=ALU.add)

                    # gate strip c interleaved so PE stays busy between
                    # conv chunks (gate = sigmoid(x @ (Wq@Wgate) + b'))
                    gs = p_str.tile([128, KC, 128], F16, tag="strip")
                    nc.sync.dma_start(gs[:], WgS[:, c, :, :])
                    strip_matmuls(
                        gs, lambda rb, ps: nc.scalar.activation(
                            gate_t[:, c, rb * 512:(rb + 1) * 512], ps[:],
                            AF.Sigmoid, bias=bg_t[:, c:c + 1]))

            # ================= phase E: coupling + gate =================
            with (
                tc.tile_pool(name="p_pg", bufs=1) as p_pg,
                tc.tile_pool(name="p_ev2", bufs=3) as p_ev2,
                tc.tile_pool(name="p_wo", bufs=1) as p_wo,
                tc.tile_pool(name="p_fw", bufs=3) as p_fw,
            ):
                wo_all = p_wo.tile([128, KC, D], F16, tag="wo")
                nc.sync.dma_start(wo_all[:], WoT[:])
                bout_t = p_wo.tile([128, D], F32, tag="bout")
                nc.sync.dma_start(bout_t[:], boutB[:])
                pg = p_pg.tile([128, KC, SEQ], F16, tag="pg")
                for co in range(KC):
                    for sb in range(4):
                        psc2 = psum_tile("ps0", 1)
                        nc.tensor.matmul(
                            psc2[:], mcB_t[:],
                            field[:, co, sb * 512:(sb + 1) * 512],
                            start=True, stop=True)
                        cpl = p_ev2.tile([128, 512], F16, tag="cpl")
                        nc.scalar.activation(cpl[:], psc2[:], AF.Identity)
                        nc.vector.scalar_tensor_tensor(
                            pg[:, co, sb * 512:(sb + 1) * 512],
                            gate_t[:, co, sb * 512:(sb + 1) * 512], 1.0,
                            cpl[:], op0=ALU.mult, op1=ALU.mult)

                # ---- phase F: out = pg @ Wout + bout ----
                for st in range(SEQ // 128):
                    pso = [psum_tile(f"ps{cb}", 1) for cb in range(2)]
                    for k in range(KC):
                        for cb in range(2):
                            nc.tensor.matmul(
                                pso[cb][:],
                                pg[:, k, st * 128:(st + 1) * 128],
                                wo_all[:, k, cb * 512:(cb + 1) * 512],
                                start=(k == 0), stop=(k == KC - 1))
                    outb = p_fw.tile([128, D], F32, tag="outb")
                    for cb in range(2):
                        nc.vector.tensor_add(
                            outb[:, cb * 512:(cb + 1) * 512], pso[cb][:],
                            bout_t[:, cb * 512:(cb + 1) * 512])
                    nc.sync.dma_start(out[st * 128:(st + 1) * 128, :],
                                      outb[:])

    nc.compile()
    _PROGRAM_CACHE["p"] = nc
    return nc


def _softmax(a, axis):
    a = a - a.max(axis=axis, keepdims=True)
    e = np.exp(a)
    return e / e.sum(axis=axis, keepdims=True)


def _host_prep(inputs):
    """Build per-core and replicated input tensors from full inputs."""
    x = np.asarray(inputs["x"], np.float32)
    Wqkv = np.asarray(inputs["Wqkv"], np.float32)
    bqkv = np.asarray(inputs["bqkv"], np.float32)
    Wout = np.asarray(inputs["Wout"], np.float32)
    bout = np.asarray(inputs["bout"], np.float32)
    Wgate = np.asarray(inputs["Wgate"], np.float32)
    bgate = np.asarray(inputs["bgate"], np.float32)
    scale_gain = np.asarray(inputs["scale_gain"], np.float64)
    skip_w = np.asarray(inputs["skip_w"], np.float64)
    coupling = np.asarray(inputs["coupling"], np.float64)

    gains = _softmax(scale_gain, axis=0)              # [11, H]
    sw = 1.0 / (1.0 + np.exp(-skip_w))                # [2]
    coup = _softmax(coupling, axis=-1)                # [H, H]

    sidx = {s: i for i, s in enumerate(SHIFTS)}
    wtab = np.zeros((NT, H), np.float64)
    for j in range(N_SCALES):
        d = 1 << j
        for t in range(4):
            wtab[sidx[(3 - t) * d]] += D4[t] * gains[j]
    ch = np.arange(D)
    wchan = np.zeros((128, KC, NT), np.float32)
    for c in range(KC):
        heads = (ch[c * 128:(c + 1) * 128] % 16)   # d-major: head = ch' % 16
        wchan[:, c, :] = wtab[:, heads].T.astype(np.float32)

    # block-diag coupling in d-major basis: mcB[dd*16+j, dd*16+i] = coup[i, j]
    mcB = np.zeros((128, 128), np.float16)
    for dd in range(8):
        mcB[dd * 16:(dd + 1) * 16, dd * 16:(dd + 1) * 16] = coup.T.astype(np.float16)

    # fold the q projection into the gate: gate = sigmoid(x @ (Wq@Wgate) + b')
    Wq = Wqkv[:, :D].astype(np.float64)
    Wqg = (Wq @ Wgate.astype(np.float64)).astype(np.float32)
    bg_f = (bqkv[:D].astype(np.float64) @ Wgate.astype(np.float64)
            + bgate.astype(np.float64)).astype(np.float32)

    def strips(W):
        """[D, D] weight -> [128, KC(strip), KC(contract), 128] fp16."""
        # W[kc*128+p, s*128+j] -> out[p, s, kc, j]
        t = W.reshape(KC, 128, KC, 128)               # [kc, p, s, j]
        return np.ascontiguousarray(
            t.transpose(1, 2, 0, 3).astype(np.float16))

    # d-major channel permutation for the v/field pipeline:
    # new channel ch' = d*16 + h  <->  original h*64 + d
    chp = np.arange(D)
    perm = (chp % 16) * HD + chp // 16          # perm[ch'] = original index
    WkS = strips(Wqkv[:, D:2 * D])
    WvS = strips(Wqkv[:, 2 * D:3 * D][:, perm])
    WgS = strips(Wqg[:, perm])
    # Wout moving layout: [p, k, m] = Wout[k*128+p, m]
    WoT = np.ascontiguousarray(
        Wout[perm, :].reshape(KC, 128, D).transpose(1, 0, 2).astype(np.float16))

    bkT = bqkv[D:2 * D].reshape(KC, 128).T.copy()     # [128, KC]
    bvT = bqkv[2 * D:3 * D][perm].reshape(KC, 128).T.copy()
    bgT = bg_f[perm].reshape(KC, 128).T.copy()
    boutB = np.broadcast_to(bout, (128, D)).copy()
    swt = np.broadcast_to(sw.astype(np.float32), (128, 2)).copy()
    bo = np.zeros((128, KC, 16), np.float16)
    for c in range(KC):
        bo[0:64, c, 2 * c] = 1.0
        bo[64:128, c, 2 * c + 1] = 1.0
    on = np.zeros((16, 128), np.float16)
    on[np.arange(128) % 16, np.arange(128)] = 1.0

    # diagonal stationaries for the 16 PE conv taps
    wdiag = np.zeros((128, KC, 18, 128), np.float16)
    for c in range(KC):
        for ti, s in enumerate(PE_TAPS):
            si = SHIFTS.index(s)
            np.fill_diagonal(wdiag[:, c, ti, :], wchan[:, c, si])

    shared = dict(WkS=WkS, WvS=WvS, WgS=WgS, mcB_in=mcB, WoT=WoT, wdiagS=wdiag,
                  bkT=bkT, bvT=bvT, bgT=bgT, boutB=boutB, wchan=wchan,
                  swt=swt, boS=bo, on_in=on)
    in_maps = []
    for c in range(NCORES):
        b, half = c // 2, c % 2
        g0 = half * SEQ
        # xm[p, kc, n] = x[b, g0+n, kc*128+p]
        xc = x[b, g0:g0 + SEQ, :].reshape(SEQ, KC, 128)
        xm = np.ascontiguousarray(
            xc.transpose(2, 1, 0).astype(np.float16))
        m = np.full((128, 1), float(half), np.float32)
        in_maps.append(dict(xm_in=xm, mask=m, **shared))
    return in_maps


def run_cores(inputs, debug_outputs=False, trace=False):
    nc = _build_program()
    in_maps = _host_prep(inputs)
    res = run_bass_kernel_spmd(nc, in_maps, list(range(NCORES)), trace=trace)
    return res


def kernel(**inputs) -> np.ndarray:
    res = run_cores(inputs)
    out = np.empty((B, N, D), np.float32)
    for c in range(NCORES):
        b, half = c // 2, c % 2
        out[b, half * SEQ:(half + 1) * SEQ, :] = res.results[c]["out"]
    return out


# revision 13
# speedup vs baseline: 1.7743x; 1.0498x over previous
"""Trainium2 Bass kernel for CausalWaveletFieldAttention (v2, fp16 datapath).

Shapes (hardcoded): x [B=4, N=4096, D=1024], H=16 heads, HD=64.
Sharding over 8 cores: core c handles (batch b = c//2, half = c%2), i.e.
2048 contiguous sequence rows of one batch.

Per-core pipeline, all in transposed [channel, seq] layout, fp16 SBUF
datapath (output tolerance is 2e-2; fp16 keeps us ~1e-3):
  1. k/v projections on PE (fp16 matmuls, k-outer loop so each stationary
     strip is reused across 4 PSUM banks), Act evicts with Square/Identity
  2. k_mag via PE block-ones reduce + Act Sqrt + PE broadcast,
     f0 = v * k_mag fused on DVE (4x fp16 mode)
  3. pairwise fp16 AllGather of f0 halves (causal-conv history)
  4. gate = sigmoid(x @ (Wq@Wgate) + b') on PE, Act Sigmoid evict,
     kept in SBUF (no DRAM round trip)
  5. 24-tap dilated wavelet FIR entirely on DVE as scalar_tensor_tensor
     chains over [128, 3072] fp16 tiles (4x packed mode, ~800ns/tap),
     extended 1024 cols back so the d=512/1024 skip taps apply locally;
     overlaps the gate matmuls on PE
  6. head coupling as dense [1024,1024] fp16 matmul on PE; Act evicts,
     DVE multiplies by the SBUF-resident gate (4x)
  7. out = (field*gate) @ Wout + bout with pg strips as PE stationaries
     straight from SBUF (no transpose round trip)
"""

import numpy as np

import concourse.bass as bass
import concourse.mybir as mybir
import concourse.tile as tile
from concourse import bacc
from concourse.bass_utils import run_bass_kernel_spmd

F32 = mybir.dt.float32
F16 = mybir.dt.float16
BF16 = mybir.dt.bfloat16
AF = mybir.ActivationFunctionType
ALU = mybir.AluOpType

B, N, D, H, HD = 4, 4096, 1024, 16, 64
NCORES = 8
SEQ = N // 2          # 2048 rows per core
KC = D // 128         # 8 contraction chunks
CONVN = SEQ + 1024    # 3072 conv outputs (1024 extra for skip taps)
EXT = SEQ + 4096      # 6144 extended f0 buffer
D4 = [0.4829629131445341, 0.8365163037378079, 0.2241438680420134, -0.1294095225512604]
N_SCALES = 11
SPARSE_DILATIONS = (512, 1024)
SHIFTS = [0, 1, 2, 3, 4, 6, 8, 12, 16, 24, 32, 48, 64, 96, 128, 192, 256,
          384, 512, 768, 1024, 1536, 2048, 3072]
NT = len(SHIFTS)      # 24 taps
DVE_TAPS = [512, 768, 1024, 1536, 2048, 3072]
PE_TAPS = [s for s in SHIFTS if s not in DVE_TAPS]   # 18 taps

_PROGRAM_CACHE = {}


def _build_program():
    if "p" in _PROGRAM_CACHE:
        return _PROGRAM_CACHE["p"]

    nc = bacc.Bacc("TRN2", target_bir_lowering=False, debug=False,
                   num_devices=NCORES)

    # ---- parameters (per-core); weights pre-packed partition-major ----
    xm_in = nc.declare_dram_parameter("xm_in", [128, KC, SEQ], F16, isOutput=False)
    mask = nc.declare_dram_parameter("mask", [128, 1], F32, isOutput=False)
    WkS = nc.declare_dram_parameter("WkS", [128, KC, KC, 128], F16, isOutput=False)
    WvS = nc.declare_dram_parameter("WvS", [128, KC, KC, 128], F16, isOutput=False)
    WgS = nc.declare_dram_parameter("WgS", [128, KC, KC, 128], F16, isOutput=False)
    mcB_in = nc.declare_dram_parameter("mcB_in", [128, 128], F16, isOutput=False)
    WoT = nc.declare_dram_parameter("WoT", [128, KC, D], F16, isOutput=False)
    wdiagS = nc.declare_dram_parameter("wdiagS", [128, KC, 18, 128], F16,
                                       isOutput=False)
    bkT = nc.declare_dram_parameter("bkT", [128, KC], F32, isOutput=False)
    bvT = nc.declare_dram_parameter("bvT", [128, KC], F32, isOutput=False)
    bgT = nc.declare_dram_parameter("bgT", [128, KC], F32, isOutput=False)
    boutB = nc.declare_dram_parameter("boutB", [128, D], F32, isOutput=False)
    wchan = nc.declare_dram_parameter("wchan", [128, KC, NT], F32, isOutput=False)
    swt = nc.declare_dram_parameter("swt", [128, 2], F32, isOutput=False)
    boS = nc.declare_dram_parameter("boS", [128, KC, 16], F16, isOutput=False)
    on_in = nc.declare_dram_parameter("on_in", [16, 128], F16, isOutput=False)
    out = nc.declare_dram_parameter("out", [SEQ, D], F32, isOutput=True)

    # ---- internal DRAM (collective staging) ----
    f0_dram = [nc.dram_tensor(f"f0_dram{c}", [128, SEQ], F16)
               for c in range(KC)]
    f0_gath = [nc.dram_tensor(f"f0_gath{c}", [2, 128, SEQ], F16)
               for c in range(KC)]

    with tile.TileContext(nc) as tc:
        with (
            tc.tile_pool(name="psum", bufs=1, space="PSUM") as psp,
            tc.tile_pool(name="const", bufs=1) as constp,
            tc.tile_pool(name="big", bufs=1) as bigp,
        ):
            # ---- constants ----
            bo_t = constp.tile([128, KC, 16], F16)
            nc.sync.dma_start(bo_t[:], boS[:])
            on_t = constp.tile([16, 128], F16)
            nc.sync.dma_start(on_t[:], on_in[:])
            mcB_t = constp.tile([128, 128], F16)
            nc.sync.dma_start(mcB_t[:], mcB_in[:])
            wchan_t = constp.tile([128, KC, NT], F32)
            nc.sync.dma_start(wchan_t[:], wchan[:])
            swt_t = constp.tile([128, 2], F32)
            nc.sync.dma_start(swt_t[:], swt[:])
            bk_t = constp.tile([128, KC], F32)
            nc.sync.dma_start(bk_t[:], bkT[:])
            bv_t = constp.tile([128, KC], F32)
            nc.sync.dma_start(bv_t[:], bvT[:])
            bg_t = constp.tile([128, KC], F32)
            nc.sync.dma_start(bg_t[:], bgT[:])
            mask_t = constp.tile([128, 1], F32)
            nc.sync.dma_start(mask_t[:], mask[:])

            gate_t = bigp.tile([128, KC, SEQ], F16, tag="gate")
            field = bigp.tile([128, KC, SEQ], F16, tag="field")

            def psum_tile(tag, bufs):
                return psp.tile([128, 512], F32, tag=tag, bufs=bufs,
                                name=tag)

            # ======== phase B (k/v/f0/gate) + phase C (conv), one scope
            # so conv overlaps the gate matmuls without pool aliasing ====
            with (
                tc.tile_pool(name="p_xm", bufs=1) as p_xm,
                tc.tile_pool(name="p_str", bufs=3) as p_str,
                tc.tile_pool(name="p_ev", bufs=2) as p_ev,
                tc.tile_pool(name="p_cw", bufs=1) as p_cw,
            ):
                xm = p_xm.tile([128, KC, SEQ], F16, tag="xm")
                ks_pre = p_str.tile([128, KC, 128], F16, tag="strip")
                nc.sync.dma_start(ks_pre[:], WkS[:, 0, :, :])
                for k in range(KC):
                    nc.sync.dma_start(xm[:, k, :], xm_in[:, k, :])

                def strip_matmuls(sr, evict):
                    """8 stationary chunks x 4 psum banks; evict(rb, psum)."""
                    pss = [psum_tile(f"ps{rb}", 1) for rb in range(4)]
                    for k in range(KC):
                        for rb in range(4):
                            nc.tensor.matmul(
                                pss[rb][:], sr[:, k, :],
                                xm[:, k, rb * 512:(rb + 1) * 512],
                                start=(k == 0), stop=(k == KC - 1))
                    for rb in range(4):
                        evict(rb, pss[rb])

                # k strips (h-major); km^2 accumulates across chunks in
                # 4 PSUM tiles [16,512] (rows 2c,2c+1 per chunk via boS)
                kmsb = [psp.tile([16, 512], F32, tag=t, bufs=b, name=t)
                        for t, b in (("cv", 2), ("cv", 2), ("psb", 1), ("km2", 1))]
                for c in range(KC):
                    if c == 0:
                        ks = ks_pre
                    else:
                        ks = p_str.tile([128, KC, 128], F16, tag="strip")
                        nc.sync.dma_start(ks[:], WkS[:, c, :, :])
                    k2b = p_ev.tile([128, SEQ], F16, tag="k2b", bufs=1)
                    strip_matmuls(
                        ks, lambda rb, ps: nc.scalar.activation(
                            k2b[:, rb * 512:(rb + 1) * 512], ps[:],
                            AF.Square, bias=bk_t[:, c:c + 1]))
                    for sb in range(4):
                        nc.tensor.matmul(kmsb[sb][:], bo_t[:, c, :],
                                         k2b[:, sb * 512:(sb + 1) * 512],
                                         start=(c == 0), stop=(c == KC - 1),
                                         skip_group_check=True)
                km_all = p_ev.tile([16, SEQ], F16, tag="km", bufs=1)
                for sb in range(4):
                    nc.scalar.activation(km_all[:, sb * 512:(sb + 1) * 512],
                                         kmsb[sb][:], AF.Sqrt)
                kmagb = p_ev.tile([128, SEQ], F16, tag="kmagb", bufs=1)
                # v strips (d-major) -> f0, store, pairwise gather
                for c in range(KC):
                    vs = p_str.tile([128, KC, 128], F16, tag="strip")
                    nc.sync.dma_start(vs[:], WvS[:, c, :, :])
                    vTb = p_ev.tile([128, SEQ], F16, tag="vTb")
                    strip_matmuls(
                        vs, lambda rb, ps: nc.scalar.activation(
                            vTb[:, rb * 512:(rb + 1) * 512], ps[:],
                            AF.Identity, bias=bv_t[:, c:c + 1]))
                    if c == 0:
                        # k_mag broadcast: identical for every d-major chunk
                        for sb in range(4):
                            pse = psum_tile("psb", 1)
                            nc.tensor.matmul(
                                pse[:], on_t[:],
                                km_all[:, sb * 512:(sb + 1) * 512],
                                start=True, stop=True)
                            nc.scalar.activation(
                                kmagb[:, sb * 512:(sb + 1) * 512], pse[:],
                                AF.Identity)
                    f0b = p_ev.tile([128, SEQ], F16, tag="f0b")
                    nc.vector.scalar_tensor_tensor(
                        f0b[:], vTb[:], 1.0, kmagb[:],
                        op0=ALU.mult, op1=ALU.mult)
                    nc.sync.dma_start(f0_dram[c][:], f0b[:])
                    nc.gpsimd.collective_compute(
                        "AllGather", ALU.bypass,
                        replica_groups=[[0, 1], [2, 3], [4, 5], [6, 7]],
                        ins=[f0_dram[c][:]], outs=[f0_gath[c][:]])

                # ---- phase C: wavelet FIR + skips (PE+DVE split) ----
                # 16 taps run on PE as diagonal matmuls (host-built diag
                # stationaries) accumulating in 2 rotating PSUM banks with
                # Act eviction; 8 taps chain on DVE; DVE merges + skips.
                exts = []
                for i in range(2):
                    e = p_cw.tile([128, EXT], F16, tag=f"ext{i}")
                    nc.vector.memset(e[:, 0:2048], 0.0)
                    exts.append(e)
                acc = [p_cw.tile([128, CONVN], F16, tag=f"acc{j}", name=f"acc{j}")
                       for j in range(2)]
                tmp = p_cw.tile([128, SEQ], F16, tag="tmp")
                for c in range(KC):
                    ext = exts[c % 2]
                    halo = p_cw.tile([128, SEQ], F16, tag="halo", bufs=2)
                    nc.sync.dma_start(halo[:], f0_gath[c][0, :, :])
                    nc.vector.tensor_scalar_mul(ext[:, 2048:4096], halo[:],
                                                mask_t[:, 0:1])
                    nc.sync.dma_start(ext[:, 4096:EXT], f0_dram[c][:])
                    wd = p_cw.tile([128, 18, 128], F16, tag="wd", bufs=2)
                    nc.sync.dma_start(wd[:], wdiagS[:, c, :, :])
                    # PE taps: per 512-block, accumulate 16 diag matmuls
                    convP = p_cw.tile([128, CONVN], F16, tag="convP", bufs=1)
                    for ob in range(CONVN // 512):
                        psc = psum_tile("cv", 2)
                        for ti, s in enumerate(PE_TAPS):
                            off = 3072 + ob * 512 - s
                            nc.tensor.matmul(psc[:], wd[:, ti, :],
                                             ext[:, off:off + 512],
                                             start=(ti == 0),
                                             stop=(ti == len(PE_TAPS) - 1))
                        nc.scalar.activation(
                            convP[:, ob * 512:(ob + 1) * 512], psc[:],
                            AF.Identity)
                    # DVE taps chain
                    cur = None
                    for ti, s in enumerate(DVE_TAPS):
                        si = SHIFTS.index(s)
                        src_ap = ext[:, 3072 - s:3072 - s + CONVN]
                        w = wchan_t[:, c, si:si + 1]
                        if cur is None:
                            cur = acc[0]
                            nc.vector.tensor_scalar_mul(cur[:], src_ap, w)
                        else:
                            nxt = acc[ti % 2]
                            nc.vector.scalar_tensor_tensor(
                                nxt[:], src_ap, w, cur[:],
                                op0=ALU.mult, op1=ALU.add)
                            cur = nxt
                    # merge PE partial: other acc buffer = convP + cur
                    mrg = acc[(DVE_TAPS.index(DVE_TAPS[-1]) + 1) % 2]
                    nc.vector.scalar_tensor_tensor(
                        mrg[:], convP[:], 1.0, cur[:],
                        op0=ALU.mult, op1=ALU.add)
                    # skips: field[n] = conv[n] + sw0*conv[n-512] + sw1*conv[n-1024]
                    nc.vector.scalar_tensor_tensor(
                        tmp[:], mrg[:, 512:512 + SEQ], swt_t[:, 0:1],
                        mrg[:, 1024:1024 + SEQ],
                        op0=ALU.mult, op1=ALU.add)
                    nc.vector.scalar_tensor_tensor(
                        field[:, c, :], mrg[:, 0:SEQ], swt_t[:, 1:2],
                        tmp[:],
                        op0=ALU.mult, op1=ALU.add)

                    # gate strip c interleaved so PE stays busy between
                    # conv chunks (gate = sigmoid(x @ (Wq@Wgate) + b'))
                    gs = p_str.tile([128, KC, 128], F16, tag="strip")
                    nc.sync.dma_start(gs[:], WgS[:, c, :, :])
                    strip_matmuls(
                        gs, lambda rb, ps: nc.scalar.activation(
                            gate_t[:, c, rb * 512:(rb + 1) * 512], ps[:],
                            AF.Sigmoid, bias=bg_t[:, c:c + 1]))

            # ================= phase E: coupling + gate =================
            with (
                tc.tile_pool(name="p_pg", bufs=1) as p_pg,
                tc.tile_pool(name="p_ev2", bufs=3) as p_ev2,
                tc.tile_pool(name="p_wo", bufs=1) as p_wo,
                tc.tile_pool(name="p_fw", bufs=3) as p_fw,
            ):
                wo_all = p_wo.tile([128, KC, D], F16, tag="wo")
                nc.sync.dma_start(wo_all[:], WoT[:])
                bout_t = p_wo.tile([128, D], F32, tag="bout")
                nc.sync.dma_start(bout_t[:], boutB[:])
                pg = p_pg.tile([128, KC, SEQ], F16, tag="pg")
                for co in range(KC):
                    for sb in range(4):
                        psc2 = psum_tile("ps0", 1)
                        nc.tensor.matmul(
                            psc2[:], mcB_t[:],
                            field[:, co, sb * 512:(sb + 1) * 512],
                            start=True, stop=True)
                        cpl = p_ev2.tile([128, 512], F16, tag="cpl")
                        nc.scalar.activation(cpl[:], psc2[:], AF.Identity)
                        nc.vector.scalar_tensor_tensor(
                            pg[:, co, sb * 512:(sb + 1) * 512],
                            gate_t[:, co, sb * 512:(sb + 1) * 512], 1.0,
                            cpl[:], op0=ALU.mult, op1=ALU.mult)

                # ---- phase F: out = pg @ Wout + bout ----
                for st in range(SEQ // 128):
                    pso = [psum_tile(f"ps{cb}", 1) for cb in range(2)]
                    for k in range(KC):
                        for cb in range(2):
                            nc.tensor.matmul(
                                pso[cb][:],
                                pg[:, k, st * 128:(st + 1) * 128],
                                wo_all[:, k, cb * 512:(cb + 1) * 512],
                                start=(k == 0), stop=(k == KC - 1))
                    outb = p_fw.tile([128, D], F32, tag="outb")
                    for cb in range(2):
                        nc.vector.tensor_add(
                            outb[:, cb * 512:(cb + 1) * 512], pso[cb][:],
                            bout_t[:, cb * 512:(cb + 1) * 512])
                    nc.sync.dma_start(out[st * 128:(st + 1) * 128, :],
                                      outb[:])

    nc.compile()
    _PROGRAM_CACHE["p"] = nc
    return nc


def _softmax(a, axis):
    a = a - a.max(axis=axis, keepdims=True)
    e = np.exp(a)
    return e / e.sum(axis=axis, keepdims=True)


def _host_prep(inputs):
    """Build per-core and replicated input tensors from full inputs."""
    x = np.asarray(inputs["x"], np.float32)
    Wqkv = np.asarray(inputs["Wqkv"], np.float32)
    bqkv = np.asarray(inputs["bqkv"], np.float32)
    Wout = np.asarray(inputs["Wout"], np.float32)
    bout = np.asarray(inputs["bout"], np.float32)
    Wgate = np.asarray(inputs["Wgate"], np.float32)
    bgate = np.asarray(inputs["bgate"], np.float32)
    scale_gain = np.asarray(inputs["scale_gain"], np.float64)
    skip_w = np.asarray(inputs["skip_w"], np.float64)
    coupling = np.asarray(inputs["coupling"], np.float64)

    gains = _softmax(scale_gain, axis=0)              # [11, H]
    sw = 1.0 / (1.0 + np.exp(-skip_w))                # [2]
    coup = _softmax(coupling, axis=-1)                # [H, H]

    sidx = {s: i for i, s in enumerate(SHIFTS)}
    wtab = np.zeros((NT, H), np.float64)
    for j in range(N_SCALES):
        d = 1 << j
        for t in range(4):
            wtab[sidx[(3 - t) * d]] += D4[t] * gains[j]
    ch = np.arange(D)
    wchan = np.zeros((128, KC, NT), np.float32)
    for c in range(KC):
        heads = (ch[c * 128:(c + 1) * 128] % 16)   # d-major: head = ch' % 16
        wchan[:, c, :] = wtab[:, heads].T.astype(np.float32)

    # block-diag coupling in d-major basis: mcB[dd*16+j, dd*16+i] = coup[i, j]
    mcB = np.zeros((128, 128), np.float16)
    for dd in range(8):
        mcB[dd * 16:(dd + 1) * 16, dd * 16:(dd + 1) * 16] = coup.T.astype(np.float16)

    # fold the q projection into the gate: gate = sigmoid(x @ (Wq@Wgate) + b')
    Wq = Wqkv[:, :D].astype(np.float64)
    Wqg = (Wq @ Wgate.astype(np.float64)).astype(np.float32)
    bg_f = (bqkv[:D].astype(np.float64) @ Wgate.astype(np.float64)
            + bgate.astype(np.float64)).astype(np.float32)

    def strips(W):
        """[D, D] weight -> [128, KC(strip), KC(contract), 128] fp16."""
        # W[kc*128+p, s*128+j] -> out[p, s, kc, j]
        t = W.reshape(KC, 128, KC, 128)               # [kc, p, s, j]
        return np.ascontiguousarray(
            t.transpose(1, 2, 0, 3).astype(np.float16))

    # d-major channel permutation for the v/field pipeline:
    # new channel ch' = d*16 + h  <->  original h*64 + d
    chp = np.arange(D)
    perm = (chp % 16) * HD + chp // 16          # perm[ch'] = original index
    WkS = strips(Wqkv[:, D:2 * D])
    WvS = strips(Wqkv[:, 2 * D:3 * D][:, perm])
    WgS = strips(Wqg[:, perm])
    # Wout moving layout: [p, k, m] = Wout[k*128+p, m]
    WoT = np.ascontiguousarray(
        Wout[perm, :].reshape(KC, 128, D).transpose(1, 0, 2).astype(np.float16))

    bkT = bqkv[D:2 * D].reshape(KC, 128).T.copy()     # [128, KC]
    bvT = bqkv[2 * D:3 * D][perm].reshape(KC, 128).T.copy()
    bgT = bg_f[perm].reshape(KC, 128).T.copy()
    boutB = np.broadcast_to(bout, (128, D)).copy()
    swt = np.broadcast_to(sw.astype(np.float32), (128, 2)).copy()
    bo = np.zeros((128, KC, 16), np.float16)
    for c in range(KC):
        bo[0:64, c, 2 * c] = 1.0
        bo[64:128, c, 2 * c + 1] = 1.0
    on = np.zeros((16, 128), np.float16)
    on[np.arange(128) % 16, np.arange(128)] = 1.0

    # diagonal stationaries for the 16 PE conv taps
    wdiag = np.zeros((128, KC, 18, 128), np.float16)
    for c in range(KC):
        for ti, s in enumerate(PE_TAPS):
            si = SHIFTS.index(s)
            np.fill_diagonal(wdiag[:, c, ti, :], wchan[:, c, si])

    shared = dict(WkS=WkS, WvS=WvS, WgS=WgS, mcB_in=mcB, WoT=WoT, wdiagS=wdiag,
                  bkT=bkT, bvT=bvT, bgT=bgT, boutB=boutB, wchan=wchan,
                  swt=swt, boS=bo, on_in=on)
    in_maps = []
    for c in range(NCORES):
        b, half = c // 2, c % 2
        g0 = half * SEQ
        # xm[p, kc, n] = x[b, g0+n, kc*128+p]
        xc = x[b, g0:g0 + SEQ, :].reshape(SEQ, KC, 128)
        xm = np.ascontiguousarray(
            xc.transpose(2, 1, 0).astype(np.float16))
        m = np.full((128, 1), float(half), np.float32)
        in_maps.append(dict(xm_in=xm, mask=m, **shared))
    return in_maps


def run_cores(inputs, debug_outputs=False, trace=False):
    nc = _build_program()
    in_maps = _host_prep(inputs)
    res = run_bass_kernel_spmd(nc, in_maps, list(range(NCORES)), trace=trace)
    return res


def kernel(**inputs) -> np.ndarray:
    res = run_cores(inputs)
    out = np.empty((B, N, D), np.float32)
    for c in range(NCORES):
        b, half = c // 2, c % 2
        out[b, half * SEQ:(half + 1) * SEQ, :] = res.results[c]["out"]
    return out
